# revision 2
# baseline (speedup 1.0000x reference)
"""Trainium2 Bass kernel for nn_NonLocalLayer (8-core data-parallel).

Math per batch n (see reference):
  theta = st @ w_st + b_st        (256,128)  -> reinterpret (128,256)  "theta_r"
  phi   = lt @ w_lt + b_lt        (4096,128) -> reinterpret (128,4096) "phi_r"
  g     = lt @ w_g  + b_g         (4096,128) -> reinterpret (128,4096) "g_r"
  attn  = theta_r^T @ phi_r / sqrt(128); p = softmax(attn, axis=l)
  out2  = g_r @ p^T               (128,256)
  y     = relu(LN(out2) * gamma + beta)      (128,256)
  out   = y[:, :, None]*w_out + b_out        (128,256,512)

Device strategy (per core = one batch):
  - host pre-transposes AND column-permutes st/lt (ltTP[c, m*128+i] =
    ltT[c, 32*i+m]) so every phi_r/g_r block is a contiguous matmul
  - big matmuls in fp16 (1 col/cyc on PE); accumulation fp32 in PSUM
  - softmax in transposed orientation (l on partitions) without
    max-subtraction (attn bounded ~ +-8); sums via ones-matmul over
    [1,512] pairs; normalization folded in after out2 accumulation
  - attention pipeline batched 2 l-blocks per stage (wider exp/copies)
  - epilogue: y flattened to one SBUF row (DMA), then out[k, d*256+s]
    = w[k]*yflat + b[k] as K=2 matmuls (lhsT = (w,b) col block, rhs =
    (yflat, ones) rows); PSUM->SBUF copies in f16 split DVE/ACT;
    OUTPUT IS STORED fp16 (tolerance 2e-2 >> f16 rounding 5e-4),
    halving the dominant HBM write traffic; host upcasts on gather
  - PE kept at 2.4 GHz (HAM warm): dummy matmuls during input loads
    and through the LayerNorm scalar chain avoid >3.4us PE-idle
    windows that would drop the clock gate to 1.2 GHz
"""
import math
import os

import numpy as np

NB = 8          # batch == n cores
S = 256         # NUM_ST
L = 4096        # NUM_LT
C = 512         # C_ST == C_LT
D = 128         # C_LAT
INV_SQRT_D = 1.0 / math.sqrt(float(D))
LN_EPS = 1e-3

_CACHE = {}
LAST_EXEC_NS = None


def _build_program():
    import concourse.bacc as bacc
    import concourse.tile as tile
    from concourse import mybir

    dt = mybir.dt
    F32 = dt.float32
    F16 = dt.float16
    AF = mybir.ActivationFunctionType
    OP = mybir.AluOpType
    AX = mybir.AxisListType

    nc = bacc.Bacc("TRN2", target_bir_lowering=False, debug=False,
                   num_devices=NB)

    d_ltT = nc.dram_tensor("ltT", [C, L], F16, kind="ExternalInput")
    d_stT = nc.dram_tensor("stT", [C, S], F16, kind="ExternalInput")
    d_wst = nc.dram_tensor("wst", [C, D], F16, kind="ExternalInput")
    d_wlt = nc.dram_tensor("wlt", [C, D], F16, kind="ExternalInput")
    d_wg = nc.dram_tensor("wg", [C, D], F16, kind="ExternalInput")
    d_bst = nc.dram_tensor("bst", [1, D], F16, kind="ExternalInput")
    d_blt = nc.dram_tensor("blt", [D, 1], F32, kind="ExternalInput")
    d_bg = nc.dram_tensor("bg", [D, 1], F32, kind="ExternalInput")
    d_gam = nc.dram_tensor("gam", [D, S], F32, kind="ExternalInput")
    d_bet = nc.dram_tensor("bet", [D, S], F32, kind="ExternalInput")
    d_idh = nc.dram_tensor("identh", [128, 128], F16, kind="ExternalInput")
    d_wb2 = nc.dram_tensor("wb2", [2, C], F16, kind="ExternalInput")
    d_ones = nc.dram_tensor("onesr", [1, D * S], F16, kind="ExternalInput")
    d_out = nc.dram_tensor("out", [C, D * S], F16, kind="ExternalOutput")

    with tile.TileContext(nc) as tc:
        # ---------- persistent pool (lives whole kernel) ----------
        with tc.tile_pool(name="keep", bufs=1) as keep:
            identh = keep.tile([128, 128], F16, tag="identh")
            bsth = keep.tile([1, D], F16, tag="bsth")
            blt_c = keep.tile([D, 1], F32, tag="blt_c")
            bg_c = keep.tile([D, 1], F32, tag="bg_c")
            gam = keep.tile([D, S], F32, tag="gam")
            bet = keep.tile([D, S], F32, tag="bet")
            wb2 = keep.tile([2, C], F16, tag="wb2")
            rhs2 = keep.tile([2, D * S], F16, tag="rhs2")
            theta_r = keep.tile([128, S], F16, tag="theta_r")
            y_r = keep.tile([D, S], F16, tag="y_r")
            warm = keep.tile([128, 512], F16, tag="warm")

            ones_f = keep.tile([128, 1], F32, tag="ones_f")
            nc.vector.memset(ones_f[:], 1.0)
            ones_r = keep.tile([128, 1], F16, tag="ones_r")
            nc.vector.tensor_copy(ones_r[:], ones_f[:])
            orow_f = keep.tile([1, 128], F32, tag="orow_f")
            nc.vector.memset(orow_f[:], 1.0)
            orow_h = keep.tile([1, 128], F16, tag="orow_h")
            nc.vector.memset(orow_h[:], 1.0)
            nc.vector.memset(warm[:], 0.25)

            # ---------- main phase ----------
            with tc.tile_pool(name="main", bufs=1) as main:
                ltTP = [main.tile([128, L], F16, tag=f"ltT{j}", name=f"ltT{j}")
                        for j in range(4)]
                stTP = [main.tile([128, S], F16, tag=f"stT{j}",
                                  name=f"stTs{j}") for j in range(4)]
                wst = [main.tile([128, D], F16, tag=f"wst{j}", name=f"wsts{j}")
                       for j in range(4)]
                wlt = [main.tile([128, D], F16, tag=f"wlt{j}", name=f"wlts{j}")
                      for j in range(4)]
                wg = [main.tile([128, D], F16, tag=f"wg{j}", name=f"wgs{j}")
                      for j in range(4)]

                # loads round-robin over the three DMA-capable queues,
                # ordered so the attention pipeline can start ~5us in
                qs = [nc.sync, nc.scalar, nc.gpsimd]
                qstate = [0]

                def ld(dst, src):
                    qs[qstate[0] % 3].dma_start(dst, src)
                    qstate[0] += 1

                for j in range(4):
                    ld(wlt[j][:], d_wlt[128 * j:128 * (j + 1), :])
                for j in range(4):
                    ld(wg[j][:], d_wg[128 * j:128 * (j + 1), :])
                for j in range(4):
                    ld(ltTP[j][:, 0:1024],
                       d_ltT[128 * j:128 * (j + 1), 0:1024])
                for j in range(4):
                    ld(stTP[j][:], d_stT[128 * j:128 * (j + 1), :])
                for j in range(4):
                    ld(wst[j][:], d_wst[128 * j:128 * (j + 1), :])
                ld(identh[:], d_idh[:])
                ld(bsth[:], d_bst[:])
                ld(blt_c[:], d_blt[:])
                ld(bg_c[:], d_bg[:])
                for t in range(1, 4):
                    for j in range(4):
                        ld(ltTP[j][:, 1024 * t:1024 * (t + 1)],
                           d_ltT[128 * j:128 * (j + 1),
                                 1024 * t:1024 * (t + 1)])
                ld(gam[:], d_gam[:])
                ld(bet[:], d_bet[:])
                ld(wb2[:], d_wb2[:])
                ld(rhs2[1:2, :], d_ones[:])

                phiP = main.tile([D, L], F16, tag="phiP")
                gP = main.tile([D, L], F16, tag="gP")

                with tc.tile_pool(name="psL", bufs=1, space="PSUM") as psL, \
                     tc.tile_pool(name="loop", bufs=1) as lp:
                    # warm the PE clock gate while inputs stream in
                    for w in range(7):
                        pw = psL.tile([128, 512], F32, tag="att", bufs=2,
                                      name=f"pw{w}")
                        nc.tensor.matmul(pw[:], warm[:, 0:128], warm[:],
                                         start=True, stop=True)

                    p_out2 = psL.tile([D, S], F32, tag="acc")
                    p_sums = psL.tile([1, 2 * S], F32, tag="sums")

                    def emit_theta():
                        for h in range(2):
                            pth = psL.tile([128, D], F32, tag="att", bufs=2,
                                           name=f"pth{h}")
                            for j in range(4):
                                nc.tensor.matmul(
                                    pth[:],
                                    stTP[j][:, 128 * h:128 * (h + 1)],
                                    wst[j][:], start=(j == 0), stop=False)
                            nc.tensor.matmul(pth[:], orow_h[:],
                                             bsth[:], start=False, stop=True)
                            nc.vector.tensor_copy(
                                theta_r[:, 128 * h:128 * (h + 1)], pth[:])

                    def emit_slice(sl):
                        cols = slice(512 * sl, 512 * (sl + 1))
                        for di, (dst, wts, bias_t) in enumerate(
                                ((phiP, wlt, blt_c), (gP, wg, bg_c))):
                            pmm = psL.tile([D, 512], F32, tag="mm", bufs=2,
                                           name=f"pmm{sl}_{di}")
                            for j in range(4):
                                nc.tensor.matmul(pmm[:], wts[j][:],
                                                 ltTP[j][:, cols],
                                                 start=(j == 0), stop=(j == 3))
                            if di == 0:
                                nc.vector.tensor_scalar(
                                    dst[:, cols], pmm[:], bias_t[:, 0:1],
                                    None, OP.add)
                            else:
                                nc.scalar.activation(dst[:, cols], pmm[:],
                                                     AF.Identity,
                                                     bias=bias_t[:, 0:1])

                    ers = {}
                    for it in range(18):
                        if it % 2 == 0 and it // 2 < 8:
                            emit_slice(it // 2)
                        if it == 0:
                            emit_theta()
                        # stage A: transpose 2 phi blocks, attn matmuls, exp
                        if 1 <= it <= 16:
                            u = it - 1
                            ptp = psL.tile([128, 256], F16, tag="ptp", bufs=2,
                                           name=f"ptp{u}")
                            for i in range(2):
                                m = 2 * u + i
                                nc.tensor.transpose(
                                    ptp[:, 128 * i:128 * (i + 1)],
                                    phiP[:, 128 * m:128 * (m + 1)],
                                    identh[:])
                            phiR = lp.tile([128, 256], F16, tag="phiR",
                                           bufs=3, name=f"phiR{u}")
                            nc.vector.tensor_copy(phiR[:], ptp[:])
                            p_att = psL.tile([128, 512], F32, tag="att",
                                             bufs=2, name=f"patt{u}")
                            for i in range(2):
                                nc.tensor.matmul(
                                    p_att[:, 256 * i:256 * (i + 1)],
                                    phiR[:, 128 * i:128 * (i + 1)],
                                    theta_r[:], start=True, stop=True)
                            er = lp.tile([128, 512], F16, tag="er", bufs=3,
                                         name=f"er{u}")
                            nc.scalar.activation(er[:], p_att[:], AF.Exp,
                                                 scale=INV_SQRT_D)
                            ers[u] = er
                        # stage B: accumulate out2 and softmax sums
                        if 2 <= it <= 17:
                            u = it - 2
                            er = ers.pop(u)
                            for i in range(2):
                                m = 2 * u + i
                                nc.tensor.matmul(
                                    p_out2[:],
                                    gP[:, 128 * m:128 * (m + 1)],
                                    er[:, 256 * i:256 * (i + 1)],
                                    start=(m == 0), stop=(m == 31))
                            nc.tensor.matmul(p_sums[:], ones_r[:], er[:],
                                             start=(u == 0), stop=(u == 15))

                    # softmax denominators -> recip -> broadcast -> normalize
                    sums_sb = main.tile([1, 2 * S], F32, tag="sums_sb")
                    nc.scalar.activation(sums_sb[:], p_sums[:], AF.Identity)
                    zf = main.tile([1, S], F32, tag="zf")
                    nc.vector.tensor_tensor(zf[:], sums_sb[:, 0:S],
                                            sums_sb[:, S:2 * S], OP.add)
                    recip = main.tile([1, S], F32, tag="recip")
                    nc.vector.reciprocal(recip[:], zf[:])
                    p_rb = psL.tile([128, S], F32, tag="att", bufs=2,
                                    name="prb")
                    nc.tensor.matmul(p_rb[:], orow_f[:], recip[:],
                                     start=True, stop=True)
                    rb_sb = main.tile([128, S], F32, tag="rb_sb")
                    nc.scalar.activation(rb_sb[:], p_rb[:], AF.Identity)
                    combo = main.tile([D, 2 * S], F32, tag="combo")
                    nc.vector.tensor_tensor(combo[:, 0:S], p_out2[:],
                                            rb_sb[:], OP.mult)
                    nc.vector.tensor_tensor(combo[:, S:2 * S], combo[:, 0:S],
                                            combo[:, 0:S], OP.mult)

                # ---------- LayerNorm + ReLU + flatten ----------
                with tc.tile_pool(name="psN", bufs=1, space="PSUM") as psN:
                    p_s12 = psN.tile([1, 2 * S], F32, tag="s12")
                    nc.tensor.matmul(p_s12[:], ones_f[:], combo[:],
                                     start=True, stop=True)
                    s12 = main.tile([1, 2 * S], F32, tag="s12sb")
                    nc.vector.tensor_copy(s12[:], p_s12[:])
                    red = main.tile([1, 2], F32, tag="red")
                    nc.vector.reduce_sum(red[:, 0:1], s12[:, 0:S], axis=AX.X)
                    nc.vector.reduce_sum(red[:, 1:2], s12[:, S:2 * S],
                                         axis=AX.X)
                    stat = main.tile([1, 4], F32, tag="stat")
                    # mean, e2
                    nc.vector.tensor_scalar(stat[:, 0:2], red[:, 0:2],
                                            1.0 / (D * S), None, OP.mult)
                    # m2 = mean^2 ; vare = e2 - m2 + eps
                    nc.vector.tensor_tensor(stat[:, 2:3], stat[:, 0:1],
                                            stat[:, 0:1], OP.mult)
                    vare = main.tile([1, 1], F32, tag="vare")
                    nc.vector.tensor_scalar(vare[:], stat[:, 1:2],
                                            stat[:, 2:3], LN_EPS,
                                            OP.subtract, OP.add)
                    sqv = main.tile([1, 1], F32, tag="sqv")
                    nc.scalar.activation(sqv[:], vare[:], AF.Sqrt)
                    rstd = main.tile([1, 1], F32, tag="rstd")
                    nc.vector.reciprocal(rstd[:], sqv[:])
                    ms = main.tile([1, 2], F32, tag="ms")
                    nc.vector.tensor_copy(ms[:, 0:1], stat[:, 0:1])
                    nc.vector.tensor_copy(ms[:, 1:2], rstd[:])
                    p_ms = psN.tile([128, 2], F32, tag="ms2")
                    nc.tensor.matmul(p_ms[:], orow_f[:], ms[:],
                                     start=True, stop=True)
                    msb = main.tile([128, 2], F32, tag="msb")
                    nc.vector.tensor_copy(msb[:], p_ms[:])
                    t1 = main.tile([D, S], F32, tag="t1")
                    nc.vector.tensor_scalar(t1[:], combo[:, 0:S],
                                            msb[:, 0:1],
                                            msb[:, 1:2], OP.subtract, OP.mult)
                    # keep the PE clock gate warm through the scalar chain
                    p_d1 = psN.tile([128, S], F32, tag="dum", bufs=2,
                                    name="pd1")
                    nc.tensor.matmul(p_d1[:], orow_f[:], t1[0:1, :],
                                     start=True, stop=True)
                    t2 = main.tile([D, S], F32, tag="t2")
                    nc.vector.tensor_tensor(t2[:], t1[:], gam[:], OP.mult)
                    p_d2 = psN.tile([128, S], F32, tag="dum", bufs=2,
                                    name="pd2")
                    nc.tensor.matmul(p_d2[:], orow_f[:], t2[0:1, :],
                                     start=True, stop=True)
                    y = main.tile([D, S], F32, tag="y")
                    nc.vector.tensor_tensor(y[:], t2[:], bet[:], OP.add)
                    nc.vector.tensor_scalar_max(y_r[:], y[:], 0.0)
                    # flatten y (relu'd, f16) into one row: col = 256*d + s
                    nc.sync.dma_start(rhs2[0:1, :], y_r[:])

            # ---------- epilogue: out[k, 256d+s] = w[k]*y[d,s] + b[k] ------
            with tc.tile_pool(name="epi", bufs=1) as ep, \
                 tc.tile_pool(name="psE", bufs=1, space="PSUM") as psE:
                tidx = 0
                for kb in range(4):
                    lhsT = wb2[:, 128 * kb:128 * (kb + 1)]
                    for t in range(4):
                        stage = ep.tile([128, 8192], F16, tag="stage",
                                        bufs=2, name=f"st{kb}_{t}")
                        for q in range(4):
                            pko = psE.tile([128, 2048], F32, tag="pko",
                                           bufs=2, name=f"pko{kb}_{t}_{q}")
                            base = 8192 * t + 2048 * q
                            for c in range(4):
                                nc.tensor.matmul(
                                    pko[:, 512 * c:512 * (c + 1)], lhsT,
                                    rhs2[:, base + 512 * c:base + 512 * (c + 1)],
                                    start=True, stop=True)
                            sl = stage[:, 2048 * q:2048 * (q + 1)]
                            if tidx % 15 < 7:
                                nc.vector.tensor_copy(sl, pko[:])
                            else:
                                nc.scalar.activation(sl, pko[:], AF.Identity)
                            tidx += 1
                        qeng = nc.sync if (kb * 4 + t) % 2 == 0 else nc.scalar
                        qeng.dma_start(
                            d_out[128 * kb:128 * (kb + 1),
                                  8192 * t:8192 * (t + 1)], stage[:])

    nc.compile()
    return nc


def _get_program():
    if "nc" not in _CACHE:
        _CACHE["nc"] = _build_program()
    return _CACHE["nc"]


def _install_ntff_shim():
    """Provide antenv.axon_hooks (absent in this image) so trace=True can
    capture NTFF profiles through the axon .so. Best-effort."""
    import sys
    import types
    try:
        from antenv.axon_hooks import get_axon_ntff_profile_hook  # noqa
        return
    except ImportError:
        pass
    try:
        from trn_agent_boot.trn_boot import _ntff_profile_via_ctypes
        hook = _ntff_profile_via_ctypes("/opt/axon/libaxon_pjrt.so")
        mod = types.ModuleType("antenv.axon_hooks")
        state = {"h": hook}
        mod.set_axon_ntff_profile_hook = lambda h: state.__setitem__("h", h)
        mod.get_axon_ntff_profile_hook = lambda: state["h"]
        sys.modules["antenv.axon_hooks"] = mod
        import antenv
        antenv.axon_hooks = mod
    except Exception as e:  # profiling is optional
        print(f"ntff shim unavailable: {e}")


def kernel(st_feat, lt_feat, w_st, b_st, w_lt, b_lt, w_g, b_g,
           ln_gamma, ln_beta, w_out, b_out):
    from concourse.bass_utils import run_bass_kernel_spmd
    global LAST_EXEC_NS

    st_feat = np.asarray(st_feat, dtype=np.float32)
    lt_feat = np.asarray(lt_feat, dtype=np.float32)

    wst = np.asarray(w_st, np.float32).astype(np.float16)
    wlt = np.asarray(w_lt, np.float32).astype(np.float16)
    wg = np.asarray(w_g, np.float32).astype(np.float16)
    wb2 = np.stack([np.asarray(w_out, np.float32).astype(np.float16),
                    np.asarray(b_out, np.float32).astype(np.float16)])
    gam = np.ascontiguousarray(np.asarray(ln_gamma, np.float32)
                               .reshape(D, S))
    bet = np.ascontiguousarray(np.asarray(ln_beta, np.float32).reshape(D, S))
    bstv = np.asarray(b_st, np.float32).astype(np.float16).reshape(1, D)
    bltv = np.asarray(b_lt, np.float32).reshape(D, 1)
    bgv = np.asarray(b_g, np.float32).reshape(D, 1)
    identh = np.eye(128, dtype=np.float16)
    onesr = np.ones((1, D * S), np.float16)

    in_maps = []
    for n in range(NB):
        # column-permuted transposes: ltTP[c, m*128 + i] = ltT[c, 32*i + m]
        # and stTP[c, h*128 + i] = stT[c, 2*i + h]
        ltT = lt_feat[n].reshape(L, C).T.astype(np.float16)
        ltTP = np.ascontiguousarray(
            ltT.reshape(C, 128, 32).transpose(0, 2, 1).reshape(C, L))
        stT = st_feat[n].reshape(S, C).T.astype(np.float16)
        stTP = np.ascontiguousarray(
            stT.reshape(C, 128, 2).transpose(0, 2, 1).reshape(C, S))
        in_maps.append({
            "ltT": ltTP, "stT": stTP, "wst": wst, "wlt": wlt, "wg": wg,
            "wb2": wb2, "bst": bstv, "blt": bltv, "bg": bgv,
            "gam": gam, "bet": bet, "identh": identh, "onesr": onesr,
        })

    nc = _get_program()
    trace = os.environ.get("BASS_KERNEL_TRACE", "") == "1"
    if trace:
        _install_ntff_shim()
    res = run_bass_kernel_spmd(nc, in_maps, core_ids=list(range(NB)),
                               trace=trace)
    LAST_EXEC_NS = res.exec_time_ns
    _CACHE["res"] = res
    out = np.empty((NB, D, S, C), np.float32)
    for n in range(NB):
        # device layout: out_dev[k, 256*d + s]
        out[n] = res.results[n]["out"].reshape(C, D, S).transpose(1, 2, 0)
    return out.reshape(NB, D, S, 1, C)



# revision 36
# speedup vs baseline: 1.1683x; 1.1683x over previous
"""Trainium2 Bass kernel for nn_NonLocalLayer (8-core data-parallel).

Math per batch n (see reference):
  theta = st @ w_st + b_st        (256,128)  -> reinterpret (128,256)  "theta_r"
  phi   = lt @ w_lt + b_lt        (4096,128) -> reinterpret (128,4096) "phi_r"
  g     = lt @ w_g  + b_g         (4096,128) -> reinterpret (128,4096) "g_r"
  attn  = theta_r^T @ phi_r / sqrt(128); p = softmax(attn, axis=l)
  out2  = g_r @ p^T               (128,256)
  y     = relu(LN(out2) * gamma + beta)      (128,256)
  out   = y[:, :, None]*w_out + b_out        (128,256,512)

Device strategy (per core = one batch):
  - host pre-transposes AND column-permutes st/lt (ltTP[c, m*128+i] =
    ltT[c, 32*i+m]) so every phi_r/g_r block is a contiguous matmul
  - big matmuls in fp16 (1 col/cyc on PE); accumulation fp32 in PSUM
  - softmax in transposed orientation (l on partitions) without
    max-subtraction (attn bounded ~ +-8); sums via ones-matmul over
    [1,512] pairs; normalization folded in after out2 accumulation
  - attention pipeline batched 2 l-blocks per stage (wider exp/copies)
  - epilogue: y flattened to one SBUF row (DMA), then out[k, d*256+s]
    = w[k]*yflat + b[k] as K=2 matmuls (lhsT = (w,b) col block, rhs =
    (yflat, ones) rows); PSUM->SBUF copies in f16 split DVE/ACT;
    OUTPUT IS STORED fp16 (tolerance 2e-2 >> f16 rounding 5e-4),
    halving the dominant HBM write traffic; host upcasts on gather
  - PE kept at 2.4 GHz (HAM warm): dummy matmuls during input loads
    and through the LayerNorm scalar chain avoid >3.4us PE-idle
    windows that would drop the clock gate to 1.2 GHz
"""
import math
import os

import numpy as np

NB = 8          # batch == n cores
S = 256         # NUM_ST
L = 4096        # NUM_LT
C = 512         # C_ST == C_LT
D = 128         # C_LAT
INV_SQRT_D = 1.0 / math.sqrt(float(D))
LN_EPS = 1e-3

_CACHE = {}
LAST_EXEC_NS = None


def _build_program():
    import concourse.bacc as bacc
    import concourse.tile as tile
    from concourse import bass_isa
    from concourse import mybir

    dt = mybir.dt
    F32 = dt.float32
    F16 = dt.float16
    BF16 = dt.bfloat16
    AF = mybir.ActivationFunctionType
    OP = mybir.AluOpType
    AX = mybir.AxisListType

    nc = bacc.Bacc("TRN2", target_bir_lowering=False, debug=False,
                   num_devices=NB)

    d_ltT = nc.dram_tensor("ltT", [C, L], F16, kind="ExternalInput")
    d_stT = nc.dram_tensor("stT", [C, S], F16, kind="ExternalInput")
    d_wst = nc.dram_tensor("wst", [C, D], F16, kind="ExternalInput")
    d_wlt = nc.dram_tensor("wlt", [C, D], F16, kind="ExternalInput")
    d_wg = nc.dram_tensor("wg", [C, D], F16, kind="ExternalInput")
    d_bst = nc.dram_tensor("bst", [1, D], F16, kind="ExternalInput")
    d_blt = nc.dram_tensor("blt", [D, 1], F32, kind="ExternalInput")
    d_bg = nc.dram_tensor("bg", [D, 1], F32, kind="ExternalInput")
    d_gam = nc.dram_tensor("gam", [D, S], F32, kind="ExternalInput")
    d_bet = nc.dram_tensor("bet", [D, S], F32, kind="ExternalInput")
    d_idh = nc.dram_tensor("identh", [128, 128], F16, kind="ExternalInput")
    d_wcol = nc.dram_tensor("wcol", [128, 4], F32, kind="ExternalInput")
    d_bcol = nc.dram_tensor("bcol", [128, 4], F32, kind="ExternalInput")
    d_out = nc.dram_tensor("out", [C, D * S], F16, kind="ExternalOutput")

    with tile.TileContext(nc) as tc:
        # ---------- persistent pool (lives whole kernel) ----------
        with tc.tile_pool(name="keep", bufs=1) as keep:
            identh = keep.tile([128, 128], F16, tag="identh")
            bsth = keep.tile([1, D], F16, tag="bsth")
            blt_c = keep.tile([D, 1], F32, tag="blt_c")
            bg_c = keep.tile([D, 1], F32, tag="bg_c")
            gam = keep.tile([D, S], F32, tag="gam")
            bet = keep.tile([D, S], F32, tag="bet")
            wcol = keep.tile([128, 4], F32, tag="wcol")
            bcol = keep.tile([128, 4], F32, tag="bcol")
            yflat = keep.tile([1, D * S], F16, tag="yflat")
            scr8 = keep.tile([1, 8], F32, tag="scr8")
            nc.vector.memset(scr8[:], 1.0)
            theta_r = keep.tile([128, S], F16, tag="theta_r")
            y_r = keep.tile([D, S], F16, tag="y_r")
            warm = keep.tile([128, 512], F16, tag="warm")

            ones128 = keep.tile([128, 128], F16, tag="ones128")
            nc.vector.memset(ones128[:], 1.0)
            ones_f32 = keep.tile([128, 128], F32, tag="ones_f32")
            nc.vector.memset(ones_f32[:], 1.0)
            orow_h = keep.tile([1, 128], F16, tag="orow_h")
            nc.vector.memset(orow_h[:], 1.0)
            nc.vector.memset(warm[:], 0.25)
            # preload the exp ACT table set while inputs stream in
            nc.scalar.activation(scr8[:, 0:1], scr8[:, 0:1], AF.Exp)

            # ---------- main phase ----------
            with tc.tile_pool(name="main", bufs=1) as main:
                ltTP = [main.tile([128, L], F16, tag=f"ltT{j}", name=f"ltT{j}")
                        for j in range(4)]
                stTP = [main.tile([128, S], F16, tag=f"stT{j}",
                                  name=f"stTs{j}") for j in range(4)]
                wst = [main.tile([128, D], F16, tag=f"wst{j}", name=f"wsts{j}")
                       for j in range(4)]
                wlt = [main.tile([128, D], F16, tag=f"wlt{j}", name=f"wlts{j}")
                      for j in range(4)]
                wg = [main.tile([128, D], F16, tag=f"wg{j}", name=f"wgs{j}")
                      for j in range(4)]

                # loads round-robin over the two HWDGE queues (SWDGE/gpsimd
                # has ~1us setup + slow drain), ordered so the attention
                # pipeline can start ~5us in
                qs = [nc.sync, nc.scalar]
                qstate = [0]

                def ld(dst, src):
                    qs[qstate[0] % 2].dma_start(dst, src)
                    qstate[0] += 1

                for j in range(4):
                    ld(wlt[j][:], d_wlt[128 * j:128 * (j + 1), :])
                for j in range(4):
                    ld(ltTP[j][:, 0:1024],
                       d_ltT[128 * j:128 * (j + 1), 0:1024])
                for j in range(4):
                    ld(wg[j][:], d_wg[128 * j:128 * (j + 1), :])
                for j in range(4):
                    ld(stTP[j][:], d_stT[128 * j:128 * (j + 1), :])
                for j in range(4):
                    ld(wst[j][:], d_wst[128 * j:128 * (j + 1), :])
                ld(identh[:], d_idh[:])
                ld(bsth[:], d_bst[:])
                ld(blt_c[:], d_blt[:])
                ld(bg_c[:], d_bg[:])
                for t in range(1, 4):
                    for j in range(4):
                        ld(ltTP[j][:, 1024 * t:1024 * (t + 1)],
                           d_ltT[128 * j:128 * (j + 1),
                                 1024 * t:1024 * (t + 1)])
                ld(gam[:], d_gam[:])
                ld(bet[:], d_bet[:])
                ld(wcol[:], d_wcol[:])
                ld(bcol[:], d_bcol[:])

                phiP = main.tile([D, L], F16, tag="phiP")
                gP = main.tile([D, L], F16, tag="gP")

                with tc.tile_pool(name="psL", bufs=1, space="PSUM") as psL, \
                     tc.tile_pool(name="loop", bufs=1) as lp:
                    # warm the PE clock gate while inputs stream in
                    for w in range(7):
                        pw = psL.tile([128, 512], F32, tag="att", bufs=2,
                                      name=f"pw{w}")
                        nc.tensor.matmul(pw[:], warm[:, 0:128], warm[:],
                                         start=True, stop=True)

                    p_out2 = psL.tile([D, S], F32, tag="acc")
                    p_sums = psL.tile([128, 2 * S], F32, tag="sums")

                    def emit_theta():
                        for h in range(2):
                            pth = psL.tile([128, D], F32, tag="att", bufs=2,
                                           name=f"pth{h}")
                            for j in range(4):
                                nc.tensor.matmul(
                                    pth[:],
                                    stTP[j][:, 128 * h:128 * (h + 1)],
                                    wst[j][:], start=(j == 0), stop=False)
                            nc.tensor.matmul(pth[:], orow_h[:],
                                             bsth[:], start=False, stop=True)
                            nc.vector.tensor_copy(
                                theta_r[:, 128 * h:128 * (h + 1)], pth[:])

                    def emit_slice(sl):
                        cols = slice(512 * sl, 512 * (sl + 1))
                        for di, (dst, wts, bias_t) in enumerate(
                                ((phiP, wlt, blt_c), (gP, wg, bg_c))):
                            pmm = psL.tile([D, 512], F32, tag="mm", bufs=2,
                                           name=f"pmm{sl}_{di}")
                            for j in range(4):
                                nc.tensor.matmul(pmm[:], wts[j][:],
                                                 ltTP[j][:, cols],
                                                 start=(j == 0), stop=(j == 3))
                            if di == 0:
                                nc.vector.tensor_scalar(
                                    dst[:, cols], pmm[:], bias_t[:, 0:1],
                                    None, OP.add)
                            else:
                                nc.scalar.activation(dst[:, cols], pmm[:],
                                                     AF.Identity,
                                                     bias=bias_t[:, 0:1])

                    ers = {}
                    for it in range(18):
                        if it % 2 == 0 and it // 2 < 8:
                            emit_slice(it // 2)
                        if it == 0:
                            emit_theta()
                        # stage A: transpose 2 phi blocks, attn matmuls, exp
                        if 1 <= it <= 16:
                            u = it - 1
                            ptp = psL.tile([128, 256], F16, tag="ptp", bufs=2,
                                           name=f"ptp{u}")
                            for i in range(2):
                                m = 2 * u + i
                                nc.tensor.transpose(
                                    ptp[:, 128 * i:128 * (i + 1)],
                                    phiP[:, 128 * m:128 * (m + 1)],
                                    identh[:])
                            phiR = lp.tile([128, 256], F16, tag="phiR",
                                           bufs=3, name=f"phiR{u}")
                            nc.vector.tensor_copy(phiR[:], ptp[:])
                            p_att = psL.tile([128, 512], F32, tag="att",
                                             bufs=2, name=f"patt{u}")
                            for i in range(2):
                                nc.tensor.matmul(
                                    p_att[:, 256 * i:256 * (i + 1)],
                                    phiR[:, 128 * i:128 * (i + 1)],
                                    theta_r[:], start=True, stop=True)
                            er = lp.tile([128, 512], F16, tag="er", bufs=3,
                                         name=f"er{u}")
                            nc.scalar.activation(er[:], p_att[:], AF.Exp,
                                                 scale=INV_SQRT_D)
                            ers[u] = er
                        # stage B: accumulate out2 and softmax sums
                        if 2 <= it <= 17:
                            u = it - 2
                            er = ers.pop(u)
                            for i in range(2):
                                m = 2 * u + i
                                nc.tensor.matmul(
                                    p_out2[:],
                                    gP[:, 128 * m:128 * (m + 1)],
                                    er[:, 256 * i:256 * (i + 1)],
                                    start=(m == 0), stop=(m == 31))
                            nc.tensor.matmul(p_sums[:], ones128[:], er[:],
                                             start=(u == 0), stop=(u == 15))

                    # softmax denominators (pre-broadcast: ones128 sums mm
                    # already produced identical rows on all 128 partitions)
                    sums_b = main.tile([128, 2 * S], F32, tag="sums_b")
                    nc.scalar.activation(sums_b[:], p_sums[:], AF.Identity)
                    # preload the sqrt ACT table set while DVE works below
                    nc.scalar.activation(scr8[:, 1:2], scr8[:, 0:1], AF.Sqrt)
                    zf = main.tile([128, S], F32, tag="zf")
                    nc.vector.tensor_tensor(zf[:], sums_b[:, 0:S],
                                            sums_b[:, S:2 * S], OP.add)
                    recip = main.tile([128, S], F32, tag="recip")
                    nc.vector.reciprocal(recip[:], zf[:])
                    # x = out2/Z, with per-partition sums of x and x^2
                    xt = main.tile([D, S], F32, tag="xt")
                    xsq = main.tile([D, S], F32, tag="xsq")
                    rs = main.tile([128, 2], F32, tag="rs")
                    nc.vector.tensor_tensor(xt[:], p_out2[:], recip[:],
                                            OP.mult)
                    nc.vector.tensor_tensor(xsq[:], xt[:], xt[:], OP.mult)
                    nc.vector.reduce_sum(rs[:, 0:1], xt[:], axis=AX.X)
                    nc.vector.reduce_sum(rs[:, 1:2], xsq[:], axis=AX.X)
                    # LN stats, kept per-partition (no broadcasts needed);
                    # partition reduction via tiny f32 ones-matmul (every
                    # output row gets the full sum)
                    p_rsum = psL.tile([128, 2], F32, tag="att", bufs=2,
                                      name="prsum")
                    nc.tensor.matmul(p_rsum[:], ones_f32[:], rs[:],
                                     start=True, stop=True)
                    rsum = main.tile([128, 2], F32, tag="rsum")
                    nc.vector.tensor_copy(rsum[:], p_rsum[:])
                    stat2 = main.tile([128, 2], F32, tag="stat2")
                    nc.vector.tensor_scalar(stat2[:], rsum[:],
                                            1.0 / (D * S), None, OP.mult)
                    m2 = main.tile([128, 1], F32, tag="m2")
                    nc.vector.tensor_tensor(m2[:], stat2[:, 0:1],
                                            stat2[:, 0:1], OP.mult)
                    vare = main.tile([128, 1], F32, tag="vare")
                    nc.vector.tensor_scalar(vare[:], stat2[:, 1:2],
                                            m2[:], LN_EPS,
                                            OP.subtract, OP.add)
                    sqv = main.tile([128, 1], F32, tag="sqv")
                    nc.scalar.activation(sqv[:], vare[:], AF.Sqrt)
                    rstd = main.tile([128, 1], F32, tag="rstd")
                    nc.vector.reciprocal(rstd[:], sqv[:])
                    t1 = main.tile([D, S], F32, tag="t1")
                    nc.vector.tensor_scalar(t1[:], xt[:],
                                            stat2[:, 0:1],
                                            rstd[:], OP.subtract, OP.mult)
                    t2 = main.tile([D, S], F32, tag="t2")
                    nc.vector.tensor_tensor(t2[:], t1[:], gam[:], OP.mult)
                    y = main.tile([D, S], F32, tag="y")
                    nc.vector.tensor_tensor(y[:], t2[:], bet[:], OP.add)
                    nc.vector.tensor_scalar_max(y_r[:], y[:], 0.0)
                    # flatten y (relu'd, f16) into one row: col = 256*d + s
                    nc.sync.dma_start(yflat[:], y_r[:])

            # ---------- epilogue: out[k, 256d+s] = w[k]*y[d,s] + b[k] ------
            # gpsimd broadcasts y to all partitions (idle engine, no PSUM);
            # DVE tensor_scalar (4x f16 mode) + ACT activation(scale,bias)
            # produce output stages directly -- no PE, no PSUM copies
            with tc.tile_pool(name="epi", bufs=1) as ep:
                ybc = ep.tile([128, D * S], F16, tag="ybc")
                edges = [0, 2048, 4096, 6144, 8192, 12288, 16384, 20480,
                         24576, 28672, 32768]
                for a, b in zip(edges, edges[1:]):
                    nc.gpsimd.partition_broadcast(ybc[:, a:b], yflat[:, a:b])
                tidx = 0
                # t=0: 512KB mini-stages, c-major, so the first DMAs need
                # only the first broadcast chunk
                for c in range(4):
                    for kb in range(4):
                        mst = ep.tile([128, 2048], F16, tag="mst",
                                      bufs=6, name=f"mst{c}_{kb}")
                        wk = wcol[:, kb:kb + 1]
                        bk = bcol[:, kb:kb + 1]
                        src = ybc[:, 2048 * c:2048 * (c + 1)]
                        if kb == 3:
                            nc.scalar.activation(mst[:], src, AF.Identity,
                                                 bias=bk, scale=wk)
                        else:
                            nc.vector.tensor_scalar(mst[:], src,
                                                    wk, bk, OP.mult, OP.add)
                        qeng = nc.sync if tidx % 2 == 0 else nc.scalar
                        qeng.dma_start(
                            d_out[128 * kb:128 * (kb + 1),
                                  2048 * c:2048 * (c + 1)], mst[:])
                        tidx += 1
                for t in range(1, 4):
                    base = 8192 * t
                    for kb in range(4):
                        stage = ep.tile([128, 8192], F16, tag="stage",
                                        bufs=3, name=f"st{t}_{kb}")
                        wk = wcol[:, kb:kb + 1]
                        bk = bcol[:, kb:kb + 1]
                        nc.vector.tensor_scalar(
                            stage[:, 0:6144], ybc[:, base:base + 6144],
                            wk, bk, OP.mult, OP.add)
                        nc.scalar.activation(
                            stage[:, 6144:8192],
                            ybc[:, base + 6144:base + 8192],
                            AF.Identity, bias=bk, scale=wk)
                        qeng = nc.sync if tidx % 2 == 0 else nc.scalar
                        qeng.dma_start(
                            d_out[128 * kb:128 * (kb + 1),
                                  base:base + 8192], stage[:])
                        tidx += 1

    nc.compile()
    return nc


def _get_program():
    if "nc" not in _CACHE:
        _CACHE["nc"] = _build_program()
    return _CACHE["nc"]


def _install_ntff_shim():
    """Provide antenv.axon_hooks (absent in this image) so trace=True can
    capture NTFF profiles through the axon .so. Best-effort."""
    import sys
    import types
    try:
        from antenv.axon_hooks import get_axon_ntff_profile_hook  # noqa
        return
    except ImportError:
        pass
    try:
        from trn_agent_boot.trn_boot import _ntff_profile_via_ctypes
        hook = _ntff_profile_via_ctypes("/opt/axon/libaxon_pjrt.so")
        mod = types.ModuleType("antenv.axon_hooks")
        state = {"h": hook}
        mod.set_axon_ntff_profile_hook = lambda h: state.__setitem__("h", h)
        mod.get_axon_ntff_profile_hook = lambda: state["h"]
        sys.modules["antenv.axon_hooks"] = mod
        import antenv
        antenv.axon_hooks = mod
    except Exception as e:  # profiling is optional
        print(f"ntff shim unavailable: {e}")


def kernel(st_feat, lt_feat, w_st, b_st, w_lt, b_lt, w_g, b_g,
           ln_gamma, ln_beta, w_out, b_out):
    from concourse.bass_utils import run_bass_kernel_spmd
    global LAST_EXEC_NS

    st_feat = np.asarray(st_feat, dtype=np.float32)
    lt_feat = np.asarray(lt_feat, dtype=np.float32)

    wst = np.asarray(w_st, np.float32).astype(np.float16)
    wlt = np.asarray(w_lt, np.float32).astype(np.float16)
    wg = np.asarray(w_g, np.float32).astype(np.float16)
    wcol = np.ascontiguousarray(
        np.asarray(w_out, np.float32).reshape(4, 128).T)
    bcol = np.ascontiguousarray(
        np.asarray(b_out, np.float32).reshape(4, 128).T)
    gam = np.ascontiguousarray(np.asarray(ln_gamma, np.float32)
                               .reshape(D, S))
    bet = np.ascontiguousarray(np.asarray(ln_beta, np.float32).reshape(D, S))
    bstv = np.asarray(b_st, np.float32).astype(np.float16).reshape(1, D)
    bltv = np.asarray(b_lt, np.float32).reshape(D, 1)
    bgv = np.asarray(b_g, np.float32).reshape(D, 1)
    identh = np.eye(128, dtype=np.float16)

    in_maps = []
    for n in range(NB):
        # column-permuted transposes: ltTP[c, m*128 + i] = ltT[c, 32*i + m]
        # and stTP[c, h*128 + i] = stT[c, 2*i + h]
        ltT = lt_feat[n].reshape(L, C).T.astype(np.float16)
        ltTP = np.ascontiguousarray(
            ltT.reshape(C, 128, 32).transpose(0, 2, 1).reshape(C, L))
        stT = st_feat[n].reshape(S, C).T.astype(np.float16)
        stTP = np.ascontiguousarray(
            stT.reshape(C, 128, 2).transpose(0, 2, 1).reshape(C, S))
        in_maps.append({
            "ltT": ltTP, "stT": stTP, "wst": wst, "wlt": wlt, "wg": wg,
            "wcol": wcol, "bcol": bcol, "bst": bstv, "blt": bltv, "bg": bgv,
            "gam": gam, "bet": bet, "identh": identh,
        })

    nc = _get_program()
    trace = os.environ.get("BASS_KERNEL_TRACE", "") == "1"
    if trace:
        _install_ntff_shim()
    res = run_bass_kernel_spmd(nc, in_maps, core_ids=list(range(NB)),
                               trace=trace)
    LAST_EXEC_NS = res.exec_time_ns
    _CACHE["res"] = res
    out = np.empty((NB, D, S, C), np.float32)
    for n in range(NB):
        # device layout: out_dev[k, 256*d + s]
        out[n] = res.results[n]["out"].reshape(C, D, S).transpose(1, 2, 0)
    return out.reshape(NB, D, S, 1, C)



# revision 39
# speedup vs baseline: 1.1997x; 1.0269x over previous
"""Trainium2 Bass kernel for nn_NonLocalLayer (8-core data-parallel).

Math per batch n (see reference):
  theta = st @ w_st + b_st        (256,128)  -> reinterpret (128,256)  "theta_r"
  phi   = lt @ w_lt + b_lt        (4096,128) -> reinterpret (128,4096) "phi_r"
  g     = lt @ w_g  + b_g         (4096,128) -> reinterpret (128,4096) "g_r"
  attn  = theta_r^T @ phi_r / sqrt(128); p = softmax(attn, axis=l)
  out2  = g_r @ p^T               (128,256)
  y     = relu(LN(out2) * gamma + beta)      (128,256)
  out   = y[:, :, None]*w_out + b_out        (128,256,512)

Device strategy (per core = one batch):
  - host pre-transposes AND column-permutes st/lt (ltTP[c, m*128+i] =
    ltT[c, 32*i+m]) so every phi_r/g_r block is a contiguous matmul
  - big matmuls in fp16 (1 col/cyc on PE); accumulation fp32 in PSUM
  - softmax in transposed orientation (l on partitions) without
    max-subtraction (attn bounded ~ +-8); sums via ones-matmul over
    [1,512] pairs; normalization folded in after out2 accumulation
  - attention pipeline batched 2 l-blocks per stage (wider exp/copies)
  - epilogue: y flattened to one SBUF row (DMA), then out[k, d*256+s]
    = w[k]*yflat + b[k] as K=2 matmuls (lhsT = (w,b) col block, rhs =
    (yflat, ones) rows); PSUM->SBUF copies in f16 split DVE/ACT;
    OUTPUT IS STORED fp16 (tolerance 2e-2 >> f16 rounding 5e-4),
    halving the dominant HBM write traffic; host upcasts on gather
  - PE kept at 2.4 GHz (HAM warm): dummy matmuls during input loads
    and through the LayerNorm scalar chain avoid >3.4us PE-idle
    windows that would drop the clock gate to 1.2 GHz
"""
import math
import os

import numpy as np

NB = 8          # batch == n cores
S = 256         # NUM_ST
L = 4096        # NUM_LT
C = 512         # C_ST == C_LT
D = 128         # C_LAT
INV_SQRT_D = 1.0 / math.sqrt(float(D))
LN_EPS = 1e-3

_CACHE = {}
LAST_EXEC_NS = None


def _build_program():
    import concourse.bacc as bacc
    import concourse.tile as tile
    from concourse import bass_isa
    from concourse import mybir

    dt = mybir.dt
    F32 = dt.float32
    F16 = dt.float16
    BF16 = dt.bfloat16
    AF = mybir.ActivationFunctionType
    OP = mybir.AluOpType
    AX = mybir.AxisListType

    nc = bacc.Bacc("TRN2", target_bir_lowering=False, debug=False,
                   num_devices=NB)

    d_ltT = nc.dram_tensor("ltT", [C, L], F16, kind="ExternalInput")
    d_stT = nc.dram_tensor("stT", [C, S], F16, kind="ExternalInput")
    d_wst = nc.dram_tensor("wst", [C, D], F16, kind="ExternalInput")
    d_wlt = nc.dram_tensor("wlt", [C, D], F16, kind="ExternalInput")
    d_wg = nc.dram_tensor("wg", [C, D], F16, kind="ExternalInput")
    d_bst = nc.dram_tensor("bst", [1, D], F16, kind="ExternalInput")
    d_blt = nc.dram_tensor("blt", [D, 1], F32, kind="ExternalInput")
    d_bg = nc.dram_tensor("bg", [D, 1], F32, kind="ExternalInput")
    d_gam = nc.dram_tensor("gam", [D, S], F32, kind="ExternalInput")
    d_bet = nc.dram_tensor("bet", [D, S], F32, kind="ExternalInput")
    d_idh = nc.dram_tensor("identh", [128, 128], F16, kind="ExternalInput")
    d_wcol = nc.dram_tensor("wcol", [128, 4], F32, kind="ExternalInput")
    d_bcol = nc.dram_tensor("bcol", [128, 4], F32, kind="ExternalInput")
    d_out = nc.dram_tensor("out", [C, D * S], BF16, kind="ExternalOutput")

    with tile.TileContext(nc) as tc:
        # ---------- persistent pool (lives whole kernel) ----------
        with tc.tile_pool(name="keep", bufs=1) as keep:
            identh = keep.tile([128, 128], F16, tag="identh")
            bsth = keep.tile([1, D], F16, tag="bsth")
            blt_c = keep.tile([D, 1], F32, tag="blt_c")
            bg_c = keep.tile([D, 1], F32, tag="bg_c")
            gam = keep.tile([D, S], F32, tag="gam")
            bet = keep.tile([D, S], F32, tag="bet")
            wcol = keep.tile([128, 4], F32, tag="wcol")
            bcol = keep.tile([128, 4], F32, tag="bcol")
            yflat = keep.tile([1, D * S], BF16, tag="yflat")
            scr8 = keep.tile([1, 8], F32, tag="scr8")
            nc.vector.memset(scr8[:], 1.0)
            theta_r = keep.tile([128, S], F16, tag="theta_r")
            y_r = keep.tile([D, S], BF16, tag="y_r")
            warm = keep.tile([128, 512], F16, tag="warm")

            ones128 = keep.tile([128, 128], F16, tag="ones128")
            nc.vector.memset(ones128[:], 1.0)
            ones_f32 = keep.tile([128, 128], F32, tag="ones_f32")
            nc.vector.memset(ones_f32[:], 1.0)
            orow_h = keep.tile([1, 128], F16, tag="orow_h")
            nc.vector.memset(orow_h[:], 1.0)
            nc.vector.memset(warm[:], 0.25)
            # preload the exp ACT table set while inputs stream in
            nc.scalar.activation(scr8[:, 0:1], scr8[:, 0:1], AF.Exp)

            # ---------- main phase ----------
            with tc.tile_pool(name="main", bufs=1) as main:
                ltTP = [main.tile([128, L], F16, tag=f"ltT{j}", name=f"ltT{j}")
                        for j in range(4)]
                stTP = [main.tile([128, S], F16, tag=f"stT{j}",
                                  name=f"stTs{j}") for j in range(4)]
                wst = [main.tile([128, D], F16, tag=f"wst{j}", name=f"wsts{j}")
                       for j in range(4)]
                wlt = [main.tile([128, D], F16, tag=f"wlt{j}", name=f"wlts{j}")
                      for j in range(4)]
                wg = [main.tile([128, D], F16, tag=f"wg{j}", name=f"wgs{j}")
                      for j in range(4)]

                # loads round-robin over the two HWDGE queues (SWDGE/gpsimd
                # has ~1us setup + slow drain), ordered so the attention
                # pipeline can start ~5us in
                qs = [nc.sync, nc.scalar]
                qstate = [0]

                def ld(dst, src):
                    qs[qstate[0] % 2].dma_start(dst, src)
                    qstate[0] += 1

                for j in range(4):
                    ld(wlt[j][:], d_wlt[128 * j:128 * (j + 1), :])
                for j in range(4):
                    ld(ltTP[j][:, 0:1024],
                       d_ltT[128 * j:128 * (j + 1), 0:1024])
                for j in range(4):
                    ld(wg[j][:], d_wg[128 * j:128 * (j + 1), :])
                for j in range(4):
                    ld(stTP[j][:], d_stT[128 * j:128 * (j + 1), :])
                for j in range(4):
                    ld(wst[j][:], d_wst[128 * j:128 * (j + 1), :])
                ld(identh[:], d_idh[:])
                ld(bsth[:], d_bst[:])
                ld(blt_c[:], d_blt[:])
                ld(bg_c[:], d_bg[:])
                for t in range(1, 3):
                    for j in range(4):
                        ld(ltTP[j][:, 1536 * t - 512:1536 * (t + 1) - 512],
                           d_ltT[128 * j:128 * (j + 1),
                                 1536 * t - 512:1536 * (t + 1) - 512])
                ld(gam[:], d_gam[:])
                ld(bet[:], d_bet[:])
                ld(wcol[:], d_wcol[:])
                ld(bcol[:], d_bcol[:])

                phiP = main.tile([D, L], F16, tag="phiP")
                gP = main.tile([D, L], F16, tag="gP")

                with tc.tile_pool(name="psL", bufs=1, space="PSUM") as psL, \
                     tc.tile_pool(name="loop", bufs=1) as lp:
                    # warm the PE clock gate while inputs stream in
                    for w in range(7):
                        pw = psL.tile([128, 512], F32, tag="att", bufs=2,
                                      name=f"pw{w}")
                        nc.tensor.matmul(pw[:], warm[:, 0:128], warm[:],
                                         start=True, stop=True)

                    p_out2 = psL.tile([D, S], F32, tag="acc")
                    p_sums = psL.tile([128, 2 * S], F32, tag="sums")

                    def emit_theta():
                        for h in range(2):
                            pth = psL.tile([128, D], F32, tag="att", bufs=2,
                                           name=f"pth{h}")
                            for j in range(4):
                                nc.tensor.matmul(
                                    pth[:],
                                    stTP[j][:, 128 * h:128 * (h + 1)],
                                    wst[j][:], start=(j == 0), stop=False)
                            nc.tensor.matmul(pth[:], orow_h[:],
                                             bsth[:], start=False, stop=True)
                            nc.vector.tensor_copy(
                                theta_r[:, 128 * h:128 * (h + 1)], pth[:])

                    def emit_slice(sl):
                        cols = slice(512 * sl, 512 * (sl + 1))
                        for di, (dst, wts, bias_t) in enumerate(
                                ((phiP, wlt, blt_c), (gP, wg, bg_c))):
                            pmm = psL.tile([D, 512], F32, tag="mm", bufs=2,
                                           name=f"pmm{sl}_{di}")
                            for j in range(4):
                                nc.tensor.matmul(pmm[:], wts[j][:],
                                                 ltTP[j][:, cols],
                                                 start=(j == 0), stop=(j == 3))
                            if di == 0:
                                nc.vector.tensor_scalar(
                                    dst[:, cols], pmm[:], bias_t[:, 0:1],
                                    None, OP.add)
                            else:
                                nc.scalar.activation(dst[:, cols], pmm[:],
                                                     AF.Identity,
                                                     bias=bias_t[:, 0:1])

                    ers = {}
                    for it in range(18):
                        if it % 2 == 0 and it // 2 < 8:
                            emit_slice(it // 2)
                        if it == 0:
                            emit_theta()
                        # stage A: transpose 2 phi blocks, attn matmuls, exp
                        if 1 <= it <= 16:
                            u = it - 1
                            ptp = psL.tile([128, 256], F16, tag="ptp", bufs=2,
                                           name=f"ptp{u}")
                            for i in range(2):
                                m = 2 * u + i
                                nc.tensor.transpose(
                                    ptp[:, 128 * i:128 * (i + 1)],
                                    phiP[:, 128 * m:128 * (m + 1)],
                                    identh[:])
                            phiR = lp.tile([128, 256], F16, tag="phiR",
                                           bufs=3, name=f"phiR{u}")
                            nc.vector.tensor_copy(phiR[:], ptp[:])
                            p_att = psL.tile([128, 512], F32, tag="att",
                                             bufs=2, name=f"patt{u}")
                            for i in range(2):
                                nc.tensor.matmul(
                                    p_att[:, 256 * i:256 * (i + 1)],
                                    phiR[:, 128 * i:128 * (i + 1)],
                                    theta_r[:], start=True, stop=True)
                            er = lp.tile([128, 512], F16, tag="er", bufs=3,
                                         name=f"er{u}")
                            nc.scalar.activation(er[:], p_att[:], AF.Exp,
                                                 scale=INV_SQRT_D)
                            ers[u] = er
                        # stage B: accumulate out2 and softmax sums
                        if 2 <= it <= 17:
                            u = it - 2
                            er = ers.pop(u)
                            for i in range(2):
                                m = 2 * u + i
                                nc.tensor.matmul(
                                    p_out2[:],
                                    gP[:, 128 * m:128 * (m + 1)],
                                    er[:, 256 * i:256 * (i + 1)],
                                    start=(m == 0), stop=(m == 31))
                            nc.tensor.matmul(p_sums[:], ones128[:], er[:],
                                             start=(u == 0), stop=(u == 15))

                    # softmax denominators (pre-broadcast: ones128 sums mm
                    # already produced identical rows on all 128 partitions)
                    sums_b = main.tile([128, 2 * S], F32, tag="sums_b")
                    nc.scalar.activation(sums_b[:], p_sums[:], AF.Identity)
                    # preload the sqrt ACT table set while DVE works below
                    nc.scalar.activation(scr8[:, 1:2], scr8[:, 0:1], AF.Sqrt)
                    zf = main.tile([128, S], F32, tag="zf")
                    nc.vector.tensor_tensor(zf[:], sums_b[:, 0:S],
                                            sums_b[:, S:2 * S], OP.add)
                    recip = main.tile([128, S], F32, tag="recip")
                    nc.vector.reciprocal(recip[:], zf[:])
                    # x = out2/Z, with per-partition sums of x and x^2
                    xt = main.tile([D, S], F32, tag="xt")
                    xsq = main.tile([D, S], F32, tag="xsq")
                    rs = main.tile([128, 2], F32, tag="rs")
                    nc.vector.tensor_tensor(xt[:], p_out2[:], recip[:],
                                            OP.mult)
                    nc.vector.tensor_tensor(xsq[:], xt[:], xt[:], OP.mult)
                    nc.vector.reduce_sum(rs[:, 0:1], xt[:], axis=AX.X)
                    nc.vector.reduce_sum(rs[:, 1:2], xsq[:], axis=AX.X)
                    # LN stats, kept per-partition (no broadcasts needed);
                    # partition reduction via tiny f32 ones-matmul (every
                    # output row gets the full sum)
                    p_rsum = psL.tile([128, 2], F32, tag="att", bufs=2,
                                      name="prsum")
                    nc.tensor.matmul(p_rsum[:], ones_f32[:], rs[:],
                                     start=True, stop=True)
                    rsum = main.tile([128, 2], F32, tag="rsum")
                    nc.vector.tensor_copy(rsum[:], p_rsum[:])
                    stat2 = main.tile([128, 2], F32, tag="stat2")
                    nc.vector.tensor_scalar(stat2[:], rsum[:],
                                            1.0 / (D * S), None, OP.mult)
                    m2 = main.tile([128, 1], F32, tag="m2")
                    nc.vector.tensor_tensor(m2[:], stat2[:, 0:1],
                                            stat2[:, 0:1], OP.mult)
                    vare = main.tile([128, 1], F32, tag="vare")
                    nc.vector.tensor_scalar(vare[:], stat2[:, 1:2],
                                            m2[:], LN_EPS,
                                            OP.subtract, OP.add)
                    sqv = main.tile([128, 1], F32, tag="sqv")
                    nc.scalar.activation(sqv[:], vare[:], AF.Sqrt)
                    rstd = main.tile([128, 1], F32, tag="rstd")
                    nc.vector.reciprocal(rstd[:], sqv[:])
                    t1 = main.tile([D, S], F32, tag="t1")
                    nc.vector.tensor_scalar(t1[:], xt[:],
                                            stat2[:, 0:1],
                                            rstd[:], OP.subtract, OP.mult)
                    t2 = main.tile([D, S], F32, tag="t2")
                    nc.vector.tensor_tensor(t2[:], t1[:], gam[:], OP.mult)
                    y = main.tile([D, S], F32, tag="y")
                    nc.vector.tensor_tensor(y[:], t2[:], bet[:], OP.add)
                    nc.vector.tensor_scalar_max(y_r[:], y[:], 0.0)
                    # flatten y (relu'd, f16) into one row: col = 256*d + s
                    nc.sync.dma_start(yflat[:], y_r[:])

            # ---------- epilogue: out[k, 256d+s] = w[k]*y[d,s] + b[k] ------
            # gpsimd broadcasts y to all partitions (idle engine, no PSUM);
            # DVE tensor_scalar (4x f16 mode) + ACT activation(scale,bias)
            # produce output stages directly -- no PE, no PSUM copies
            with tc.tile_pool(name="epi", bufs=1) as ep:
                ybc = ep.tile([128, D * S], BF16, tag="ybc")
                edges = [0, 2048, 4096, 6144, 8192, 12288, 16384, 20480,
                         24576, 28672, 32768]
                for a, b in zip(edges, edges[1:]):
                    nc.gpsimd.partition_broadcast(ybc[:, a:b], yflat[:, a:b])
                tidx = 0
                # t=0: 512KB mini-stages, c-major, so the first DMAs need
                # only the first broadcast chunk
                for c in range(4):
                    for kb in range(4):
                        mst = ep.tile([128, 2048], BF16, tag="mst",
                                      bufs=6, name=f"mst{c}_{kb}")
                        wk = wcol[:, kb:kb + 1]
                        bk = bcol[:, kb:kb + 1]
                        src = ybc[:, 2048 * c:2048 * (c + 1)]
                        if kb % 2 == 1:
                            nc.scalar.activation(mst[:], src, AF.Identity,
                                                 bias=bk, scale=wk)
                        else:
                            nc.vector.tensor_scalar(mst[:], src,
                                                    wk, bk, OP.mult, OP.add)
                        qeng = nc.sync if tidx % 2 == 0 else nc.scalar
                        qeng.dma_start(
                            d_out[128 * kb:128 * (kb + 1),
                                  2048 * c:2048 * (c + 1)], mst[:])
                        tidx += 1
                for t in range(1, 4):
                    base = 8192 * t
                    for kb in range(4):
                        stage = ep.tile([128, 8192], BF16, tag="stage",
                                        bufs=3, name=f"st{t}_{kb}")
                        wk = wcol[:, kb:kb + 1]
                        bk = bcol[:, kb:kb + 1]
                        nc.vector.tensor_scalar(
                            stage[:, 0:4096], ybc[:, base:base + 4096],
                            wk, bk, OP.mult, OP.add)
                        nc.scalar.activation(
                            stage[:, 4096:8192],
                            ybc[:, base + 4096:base + 8192],
                            AF.Identity, bias=bk, scale=wk)
                        qeng = nc.sync if tidx % 2 == 0 else nc.scalar
                        qeng.dma_start(
                            d_out[128 * kb:128 * (kb + 1),
                                  base:base + 8192], stage[:])
                        tidx += 1

    nc.compile()
    return nc


def _get_program():
    if "nc" not in _CACHE:
        _CACHE["nc"] = _build_program()
    return _CACHE["nc"]


def _install_ntff_shim():
    """Provide antenv.axon_hooks (absent in this image) so trace=True can
    capture NTFF profiles through the axon .so. Best-effort."""
    import sys
    import types
    try:
        from antenv.axon_hooks import get_axon_ntff_profile_hook  # noqa
        return
    except ImportError:
        pass
    try:
        from trn_agent_boot.trn_boot import _ntff_profile_via_ctypes
        hook = _ntff_profile_via_ctypes("/opt/axon/libaxon_pjrt.so")
        mod = types.ModuleType("antenv.axon_hooks")
        state = {"h": hook}
        mod.set_axon_ntff_profile_hook = lambda h: state.__setitem__("h", h)
        mod.get_axon_ntff_profile_hook = lambda: state["h"]
        sys.modules["antenv.axon_hooks"] = mod
        import antenv
        antenv.axon_hooks = mod
    except Exception as e:  # profiling is optional
        print(f"ntff shim unavailable: {e}")


def kernel(st_feat, lt_feat, w_st, b_st, w_lt, b_lt, w_g, b_g,
           ln_gamma, ln_beta, w_out, b_out):
    from concourse.bass_utils import run_bass_kernel_spmd
    global LAST_EXEC_NS

    st_feat = np.asarray(st_feat, dtype=np.float32)
    lt_feat = np.asarray(lt_feat, dtype=np.float32)

    wst = np.asarray(w_st, np.float32).astype(np.float16)
    wlt = np.asarray(w_lt, np.float32).astype(np.float16)
    wg = np.asarray(w_g, np.float32).astype(np.float16)
    wcol = np.ascontiguousarray(
        np.asarray(w_out, np.float32).reshape(4, 128).T)
    bcol = np.ascontiguousarray(
        np.asarray(b_out, np.float32).reshape(4, 128).T)
    gam = np.ascontiguousarray(np.asarray(ln_gamma, np.float32)
                               .reshape(D, S))
    bet = np.ascontiguousarray(np.asarray(ln_beta, np.float32).reshape(D, S))
    bstv = np.asarray(b_st, np.float32).astype(np.float16).reshape(1, D)
    bltv = np.asarray(b_lt, np.float32).reshape(D, 1)
    bgv = np.asarray(b_g, np.float32).reshape(D, 1)
    identh = np.eye(128, dtype=np.float16)

    in_maps = []
    for n in range(NB):
        # column-permuted transposes: ltTP[c, m*128 + i] = ltT[c, 32*i + m]
        # and stTP[c, h*128 + i] = stT[c, 2*i + h]
        ltT = lt_feat[n].reshape(L, C).T.astype(np.float16)
        ltTP = np.ascontiguousarray(
            ltT.reshape(C, 128, 32).transpose(0, 2, 1).reshape(C, L))
        stT = st_feat[n].reshape(S, C).T.astype(np.float16)
        stTP = np.ascontiguousarray(
            stT.reshape(C, 128, 2).transpose(0, 2, 1).reshape(C, S))
        in_maps.append({
            "ltT": ltTP, "stT": stTP, "wst": wst, "wlt": wlt, "wg": wg,
            "wcol": wcol, "bcol": bcol, "bst": bstv, "blt": bltv, "bg": bgv,
            "gam": gam, "bet": bet, "identh": identh,
        })

    nc = _get_program()
    trace = os.environ.get("BASS_KERNEL_TRACE", "") == "1"
    if trace:
        _install_ntff_shim()
    res = run_bass_kernel_spmd(nc, in_maps, core_ids=list(range(NB)),
                               trace=trace)
    LAST_EXEC_NS = res.exec_time_ns
    _CACHE["res"] = res
    out = np.empty((NB, D, S, C), np.float32)
    for n in range(NB):
        # device layout: out_dev[k, 256*d + s]
        out[n] = res.results[n]["out"].reshape(C, D, S).transpose(1, 2, 0)
    return out.reshape(NB, D, S, 1, C)



# revision 43
# speedup vs baseline: 1.2952x; 1.0796x over previous
"""Trainium2 Bass kernel for nn_NonLocalLayer (8-core data-parallel).

Math per batch n (see reference):
  theta = st @ w_st + b_st        (256,128)  -> reinterpret (128,256)  "theta_r"
  phi   = lt @ w_lt + b_lt        (4096,128) -> reinterpret (128,4096) "phi_r"
  g     = lt @ w_g  + b_g         (4096,128) -> reinterpret (128,4096) "g_r"
  attn  = theta_r^T @ phi_r / sqrt(128); p = softmax(attn, axis=l)
  out2  = g_r @ p^T               (128,256)
  y     = relu(LN(out2) * gamma + beta)      (128,256)
  out   = y[:, :, None]*w_out + b_out        (128,256,512)

Device strategy (per core = one batch):
  - host pre-transposes AND column-permutes st/lt (ltTP[c, m*128+i] =
    ltT[c, 32*i+m]) so every phi_r/g_r block is a contiguous matmul
  - big matmuls in fp16 (1 col/cyc on PE); accumulation fp32 in PSUM
  - softmax in transposed orientation (l on partitions) without
    max-subtraction (attn bounded ~ +-8); sums via ones-matmul over
    [1,512] pairs; normalization folded in after out2 accumulation
  - attention pipeline batched 2 l-blocks per stage (wider exp/copies)
  - epilogue: y flattened to one SBUF row (DMA), then out[k, d*256+s]
    = w[k]*yflat + b[k] as K=2 matmuls (lhsT = (w,b) col block, rhs =
    (yflat, ones) rows); PSUM->SBUF copies in f16 split DVE/ACT;
    OUTPUT IS STORED fp16 (tolerance 2e-2 >> f16 rounding 5e-4),
    halving the dominant HBM write traffic; host upcasts on gather
  - PE kept at 2.4 GHz (HAM warm): dummy matmuls during input loads
    and through the LayerNorm scalar chain avoid >3.4us PE-idle
    windows that would drop the clock gate to 1.2 GHz
"""
import math
import os

import numpy as np

NB = 8          # batch == n cores
S = 256         # NUM_ST
L = 4096        # NUM_LT
C = 512         # C_ST == C_LT
D = 128         # C_LAT
INV_SQRT_D = 1.0 / math.sqrt(float(D))
LN_EPS = 1e-3

_CACHE = {}
LAST_EXEC_NS = None


def _build_program():
    import concourse.bacc as bacc
    import concourse.tile as tile
    from concourse import bass_isa
    from concourse import mybir

    dt = mybir.dt
    F32 = dt.float32
    F16 = dt.float16
    BF16 = dt.bfloat16
    AF = mybir.ActivationFunctionType
    OP = mybir.AluOpType
    AX = mybir.AxisListType

    nc = bacc.Bacc("TRN2", target_bir_lowering=False, debug=False,
                   num_devices=NB)

    d_ltT = nc.dram_tensor("ltT", [128, 4 * L], F16, kind="ExternalInput")
    d_stT = nc.dram_tensor("stT", [128, 4 * S], F16, kind="ExternalInput")
    d_wst = nc.dram_tensor("wst", [128, 4 * D], F16, kind="ExternalInput")
    d_wlt = nc.dram_tensor("wlt", [128, 4 * D], F16, kind="ExternalInput")
    d_wg = nc.dram_tensor("wg", [128, 4 * D], F16, kind="ExternalInput")
    d_bst = nc.dram_tensor("bst", [1, D], F16, kind="ExternalInput")
    d_blt = nc.dram_tensor("blt", [D, 1], F32, kind="ExternalInput")
    d_bg = nc.dram_tensor("bg", [D, 1], F32, kind="ExternalInput")
    d_gam = nc.dram_tensor("gam", [D, S], F32, kind="ExternalInput")
    d_bet = nc.dram_tensor("bet", [D, S], F32, kind="ExternalInput")
    d_idh = nc.dram_tensor("identh", [128, 128], F16, kind="ExternalInput")
    d_wcol = nc.dram_tensor("wcol", [128, 4], F32, kind="ExternalInput")
    d_bcol = nc.dram_tensor("bcol", [128, 4], F32, kind="ExternalInput")
    d_out = nc.dram_tensor("out", [C, D * S], BF16, kind="ExternalOutput")

    with tile.TileContext(nc) as tc:
        # ---------- persistent pool (lives whole kernel) ----------
        with tc.tile_pool(name="keep", bufs=1) as keep:
            identh = keep.tile([128, 128], F16, tag="identh")
            bsth = keep.tile([1, D], F16, tag="bsth")
            blt_c = keep.tile([D, 1], F32, tag="blt_c")
            bg_c = keep.tile([D, 1], F32, tag="bg_c")
            gam = keep.tile([D, S], F32, tag="gam")
            bet = keep.tile([D, S], F32, tag="bet")
            wcol = keep.tile([128, 4], F32, tag="wcol")
            bcol = keep.tile([128, 4], F32, tag="bcol")
            yflat = keep.tile([1, D * S], BF16, tag="yflat")
            scr8 = keep.tile([1, 8], F32, tag="scr8")
            nc.vector.memset(scr8[:], 1.0)
            theta_r = keep.tile([128, S], F16, tag="theta_r")
            y_r = keep.tile([D, S], BF16, tag="y_r")
            warm = keep.tile([128, 512], F16, tag="warm")

            ones128 = keep.tile([128, 128], F16, tag="ones128")
            nc.vector.memset(ones128[:], 1.0)
            ones_f32 = keep.tile([128, 128], F32, tag="ones_f32")
            nc.vector.memset(ones_f32[:], 1.0)
            orow_h = keep.tile([1, 128], F16, tag="orow_h")
            nc.vector.memset(orow_h[:], 1.0)
            nc.vector.memset(warm[:], 0.25)
            # preload the exp ACT table set while inputs stream in
            nc.scalar.activation(scr8[:, 0:1], scr8[:, 0:1], AF.Exp)

            # ---------- main phase ----------
            with tc.tile_pool(name="main", bufs=1) as main:
                # consolidated input tiles: all 4 k-blocks (j) side by
                # side, host-reordered so each 1MB DMA covers one lt
                # t-chunk for ALL j (fewer, bigger DMAs)
                ltt2 = main.tile([128, 4 * L], F16, tag="ltt2")
                stT2 = main.tile([128, 4 * S], F16, tag="stT2")
                wst2 = main.tile([128, 4 * D], F16, tag="wst2")
                wlt2 = main.tile([128, 4 * D], F16, tag="wlt2")
                wg2 = main.tile([128, 4 * D], F16, tag="wg2")

                def lt_sl(j, c0, c1):
                    # cols [c0,c1) of original ltTP[j]; c0,c1 within one
                    # 1024-col t-chunk
                    t = c0 // 1024
                    o = 4096 * t + 1024 * j + (c0 - 1024 * t)
                    return ltt2[:, o:o + (c1 - c0)]

                # loads round-robin over the two HWDGE queues (SWDGE/gpsimd
                # has ~1us setup + slow drain), ordered so the attention
                # pipeline can start ~5us in
                qs = [nc.sync, nc.scalar]
                qstate = [0]

                def ld(dst, src):
                    qs[qstate[0] % 2].dma_start(dst, src)
                    qstate[0] += 1

                ld(wlt2[:], d_wlt[:])
                ld(ltt2[:, 0:4096], d_ltT[:, 0:4096])
                ld(wg2[:], d_wg[:])
                ld(stT2[:], d_stT[:])
                ld(wst2[:], d_wst[:])
                ld(identh[:], d_idh[:])
                ld(bsth[:], d_bst[:])
                ld(blt_c[:], d_blt[:])
                ld(bg_c[:], d_bg[:])
                for t in range(1, 4):
                    ld(ltt2[:, 4096 * t:4096 * (t + 1)],
                       d_ltT[:, 4096 * t:4096 * (t + 1)])
                ld(gam[:], d_gam[:])
                ld(bet[:], d_bet[:])
                ld(wcol[:], d_wcol[:])
                ld(bcol[:], d_bcol[:])

                phiP = main.tile([D, L], F16, tag="phiP")
                gP = main.tile([D, L], F16, tag="gP")

                with tc.tile_pool(name="psL", bufs=1, space="PSUM") as psL, \
                     tc.tile_pool(name="loop", bufs=1) as lp:
                    # warm the PE clock gate while inputs stream in
                    for w in range(7):
                        pw = psL.tile([128, 512], F32, tag="att", bufs=2,
                                      name=f"pw{w}")
                        nc.tensor.matmul(pw[:], warm[:, 0:128], warm[:],
                                         start=True, stop=True)

                    p_out2 = psL.tile([D, S], F32, tag="acc")
                    p_sums = psL.tile([128, 2 * S], F32, tag="sums")

                    def emit_theta():
                        for h in range(2):
                            pth = psL.tile([128, D], F32, tag="att", bufs=2,
                                           name=f"pth{h}")
                            for j in range(4):
                                nc.tensor.matmul(
                                    pth[:],
                                    stT2[:, 256 * j + 128 * h:
                                         256 * j + 128 * (h + 1)],
                                    wst2[:, 128 * j:128 * (j + 1)],
                                    start=(j == 0), stop=False)
                            nc.tensor.matmul(pth[:], orow_h[:],
                                             bsth[:], start=False, stop=True)
                            nc.vector.tensor_copy(
                                theta_r[:, 128 * h:128 * (h + 1)], pth[:])

                    def emit_slice(sl):
                        cols = slice(512 * sl, 512 * (sl + 1))
                        for di, (dst, wts, bias_t) in enumerate(
                                ((phiP, wlt2, blt_c), (gP, wg2, bg_c))):
                            pmm = psL.tile([D, 512], F32, tag="mm", bufs=2,
                                           name=f"pmm{sl}_{di}")
                            for j in range(4):
                                nc.tensor.matmul(
                                    pmm[:],
                                    wts[:, 128 * j:128 * (j + 1)],
                                    lt_sl(j, 512 * sl, 512 * (sl + 1)),
                                    start=(j == 0), stop=(j == 3))
                            if di == 0:
                                nc.vector.tensor_scalar(
                                    dst[:, cols], pmm[:], bias_t[:, 0:1],
                                    None, OP.add)
                            else:
                                nc.scalar.activation(dst[:, cols], pmm[:],
                                                     AF.Identity,
                                                     bias=bias_t[:, 0:1])

                    ers = {}
                    for it in range(18):
                        if it % 2 == 0 and it // 2 < 8:
                            emit_slice(it // 2)
                        if it == 0:
                            emit_theta()
                        # stage A: transpose 2 phi blocks, attn matmuls, exp
                        if 1 <= it <= 16:
                            u = it - 1
                            ptp = psL.tile([128, 256], F16, tag="ptp", bufs=2,
                                           name=f"ptp{u}")
                            for i in range(2):
                                m = 2 * u + i
                                nc.tensor.transpose(
                                    ptp[:, 128 * i:128 * (i + 1)],
                                    phiP[:, 128 * m:128 * (m + 1)],
                                    identh[:])
                            phiR = lp.tile([128, 256], F16, tag="phiR",
                                           bufs=3, name=f"phiR{u}")
                            nc.vector.tensor_copy(phiR[:], ptp[:])
                            p_att = psL.tile([128, 512], F32, tag="att",
                                             bufs=2, name=f"patt{u}")
                            for i in range(2):
                                nc.tensor.matmul(
                                    p_att[:, 256 * i:256 * (i + 1)],
                                    phiR[:, 128 * i:128 * (i + 1)],
                                    theta_r[:], start=True, stop=True)
                            er = lp.tile([128, 512], F16, tag="er", bufs=3,
                                         name=f"er{u}")
                            nc.scalar.activation(er[:], p_att[:], AF.Exp,
                                                 scale=INV_SQRT_D)
                            ers[u] = er
                        # stage B: accumulate out2 and softmax sums
                        if 2 <= it <= 17:
                            u = it - 2
                            er = ers.pop(u)
                            for i in range(2):
                                m = 2 * u + i
                                nc.tensor.matmul(
                                    p_out2[:],
                                    gP[:, 128 * m:128 * (m + 1)],
                                    er[:, 256 * i:256 * (i + 1)],
                                    start=(m == 0), stop=(m == 31))
                            nc.tensor.matmul(p_sums[:], ones128[:], er[:],
                                             start=(u == 0), stop=(u == 15))

                    # softmax denominators (pre-broadcast: ones128 sums mm
                    # already produced identical rows on all 128 partitions)
                    sums_b = main.tile([128, 2 * S], F32, tag="sums_b")
                    nc.scalar.activation(sums_b[:], p_sums[:], AF.Identity)
                    # preload the sqrt ACT table set while DVE works below
                    nc.scalar.activation(scr8[:, 1:2], scr8[:, 0:1], AF.Sqrt)
                    zf = main.tile([128, S], F32, tag="zf")
                    nc.vector.tensor_tensor(zf[:], sums_b[:, 0:S],
                                            sums_b[:, S:2 * S], OP.add)
                    recip = main.tile([128, S], F32, tag="recip")
                    nc.vector.reciprocal(recip[:], zf[:])
                    # x = out2/Z, with per-partition sums of x and x^2
                    xt = main.tile([D, S], F32, tag="xt")
                    xsq = main.tile([D, S], F32, tag="xsq")
                    rs = main.tile([128, 2], F32, tag="rs")
                    nc.vector.tensor_tensor(xt[:], p_out2[:], recip[:],
                                            OP.mult)
                    nc.vector.tensor_tensor(xsq[:], xt[:], xt[:], OP.mult)
                    nc.vector.reduce_sum(rs[:, 0:1], xt[:], axis=AX.X)
                    nc.vector.reduce_sum(rs[:, 1:2], xsq[:], axis=AX.X)
                    # LN stats, kept per-partition (no broadcasts needed);
                    # partition reduction via tiny f32 ones-matmul (every
                    # output row gets the full sum)
                    p_rsum = psL.tile([128, 2], F32, tag="att", bufs=2,
                                      name="prsum")
                    nc.tensor.matmul(p_rsum[:], ones_f32[:], rs[:],
                                     start=True, stop=True)
                    rsum = main.tile([128, 2], F32, tag="rsum")
                    nc.vector.tensor_copy(rsum[:], p_rsum[:])
                    stat2 = main.tile([128, 2], F32, tag="stat2")
                    nc.vector.tensor_scalar(stat2[:], rsum[:],
                                            1.0 / (D * S), None, OP.mult)
                    m2 = main.tile([128, 1], F32, tag="m2")
                    nc.vector.tensor_tensor(m2[:], stat2[:, 0:1],
                                            stat2[:, 0:1], OP.mult)
                    vare = main.tile([128, 1], F32, tag="vare")
                    nc.vector.tensor_scalar(vare[:], stat2[:, 1:2],
                                            m2[:], LN_EPS,
                                            OP.subtract, OP.add)
                    sqv = main.tile([128, 1], F32, tag="sqv")
                    nc.scalar.activation(sqv[:], vare[:], AF.Sqrt)
                    rstd = main.tile([128, 1], F32, tag="rstd")
                    nc.vector.reciprocal(rstd[:], sqv[:])
                    t1 = main.tile([D, S], F32, tag="t1")
                    nc.vector.tensor_scalar(t1[:], xt[:],
                                            stat2[:, 0:1],
                                            rstd[:], OP.subtract, OP.mult)
                    t2 = main.tile([D, S], F32, tag="t2")
                    nc.vector.tensor_tensor(t2[:], t1[:], gam[:], OP.mult)
                    y = main.tile([D, S], F32, tag="y")
                    nc.vector.tensor_tensor(y[:], t2[:], bet[:], OP.add)
                    nc.vector.tensor_scalar_max(y_r[:], y[:], 0.0)
                    # flatten y (relu'd, f16) into one row: col = 256*d + s
                    nc.sync.dma_start(yflat[:], y_r[:])

            # ---------- epilogue: out[k, 256d+s] = w[k]*y[d,s] + b[k] ------
            # gpsimd broadcasts y to all partitions (idle engine, no PSUM);
            # DVE tensor_scalar (4x f16 mode) + ACT activation(scale,bias)
            # produce output stages directly -- no PE, no PSUM copies
            with tc.tile_pool(name="epi", bufs=1) as ep:
                ybc = ep.tile([128, D * S], BF16, tag="ybc")
                edges = [0, 2048, 4096, 6144, 8192, 12288, 16384, 20480,
                         24576, 28672, 32768]
                for a, b in zip(edges, edges[1:]):
                    nc.gpsimd.partition_broadcast(ybc[:, a:b], yflat[:, a:b])
                tidx = 0
                # t=0: 512KB mini-stages, c-major, so the first DMAs need
                # only the first broadcast chunk
                for c in range(4):
                    for kb in range(4):
                        mst = ep.tile([128, 2048], BF16, tag="mst",
                                      bufs=6, name=f"mst{c}_{kb}")
                        wk = wcol[:, kb:kb + 1]
                        bk = bcol[:, kb:kb + 1]
                        src = ybc[:, 2048 * c:2048 * (c + 1)]
                        if kb % 2 == 1:
                            nc.scalar.activation(mst[:], src, AF.Identity,
                                                 bias=bk, scale=wk)
                        elif kb == 2:
                            nc.vector.tensor_scalar_mul(mst[:], src, wk)
                            nc.vector.tensor_scalar_add(mst[:], mst[:], bk)
                        else:
                            nc.vector.tensor_scalar(mst[:], src,
                                                    wk, bk, OP.mult, OP.add)
                        qeng = nc.sync if tidx % 2 == 0 else nc.scalar
                        qeng.dma_start(
                            d_out[128 * kb:128 * (kb + 1),
                                  2048 * c:2048 * (c + 1)], mst[:])
                        tidx += 1
                for t in range(1, 4):
                    base = 8192 * t
                    for kb in range(4):
                        stage = ep.tile([128, 8192], BF16, tag="stage",
                                        bufs=3, name=f"st{t}_{kb}")
                        wk = wcol[:, kb:kb + 1]
                        bk = bcol[:, kb:kb + 1]
                        nc.vector.tensor_scalar(
                            stage[:, 0:2048], ybc[:, base:base + 2048],
                            wk, bk, OP.mult, OP.add)
                        nc.vector.tensor_scalar_mul(
                            stage[:, 2048:3072],
                            ybc[:, base + 2048:base + 3072], wk)
                        nc.vector.tensor_scalar_add(
                            stage[:, 2048:3072], stage[:, 2048:3072], bk)
                        nc.scalar.activation(
                            stage[:, 3072:8192],
                            ybc[:, base + 3072:base + 8192],
                            AF.Identity, bias=bk, scale=wk)
                        qeng = nc.sync if tidx % 2 == 0 else nc.scalar
                        qeng.dma_start(
                            d_out[128 * kb:128 * (kb + 1),
                                  base:base + 8192], stage[:])
                        tidx += 1

    nc.compile()
    return nc


def _get_program():
    if "nc" not in _CACHE:
        _CACHE["nc"] = _build_program()
    return _CACHE["nc"]


def _install_ntff_shim():
    """Provide antenv.axon_hooks (absent in this image) so trace=True can
    capture NTFF profiles through the axon .so. Best-effort."""
    import sys
    import types
    try:
        from antenv.axon_hooks import get_axon_ntff_profile_hook  # noqa
        return
    except ImportError:
        pass
    try:
        from trn_agent_boot.trn_boot import _ntff_profile_via_ctypes
        hook = _ntff_profile_via_ctypes("/opt/axon/libaxon_pjrt.so")
        mod = types.ModuleType("antenv.axon_hooks")
        state = {"h": hook}
        mod.set_axon_ntff_profile_hook = lambda h: state.__setitem__("h", h)
        mod.get_axon_ntff_profile_hook = lambda: state["h"]
        sys.modules["antenv.axon_hooks"] = mod
        import antenv
        antenv.axon_hooks = mod
    except Exception as e:  # profiling is optional
        print(f"ntff shim unavailable: {e}")


def prepare_in_maps(st_feat, lt_feat, w_st, b_st, w_lt, b_lt, w_g, b_g,
                    ln_gamma, ln_beta, w_out, b_out):
    st_feat = np.asarray(st_feat, dtype=np.float32)
    lt_feat = np.asarray(lt_feat, dtype=np.float32)

    def wpack(w):
        w = np.asarray(w, np.float32).astype(np.float16)
        return np.ascontiguousarray(
            w.reshape(4, 128, 128).transpose(1, 0, 2).reshape(128, 512))

    wst = wpack(w_st)
    wlt = wpack(w_lt)
    wg = wpack(w_g)
    wcol = np.ascontiguousarray(
        np.asarray(w_out, np.float32).reshape(4, 128).T)
    bcol = np.ascontiguousarray(
        np.asarray(b_out, np.float32).reshape(4, 128).T)
    gam = np.ascontiguousarray(np.asarray(ln_gamma, np.float32)
                               .reshape(D, S))
    bet = np.ascontiguousarray(np.asarray(ln_beta, np.float32).reshape(D, S))
    bstv = np.asarray(b_st, np.float32).astype(np.float16).reshape(1, D)
    bltv = np.asarray(b_lt, np.float32).reshape(D, 1)
    bgv = np.asarray(b_g, np.float32).reshape(D, 1)
    identh = np.eye(128, dtype=np.float16)

    in_maps = []
    for n in range(NB):
        # column-permuted transposes: ltTP[c, m*128 + i] = ltT[c, 32*i + m]
        # and stTP[c, h*128 + i] = stT[c, 2*i + h]
        ltT = lt_feat[n].reshape(L, C).T.astype(np.float16)
        ltTP = ltT.reshape(C, 128, 32).transpose(0, 2, 1).reshape(C, L)
        # pack: lt2[p, 4096t + 1024j + i] = ltTP[128j + p, 1024t + i]
        ltTP = np.ascontiguousarray(
            ltTP.reshape(4, 128, 4, 1024).transpose(1, 2, 0, 3)
            .reshape(128, 4 * L))
        stT = st_feat[n].reshape(S, C).T.astype(np.float16)
        stTP = stT.reshape(C, 128, 2).transpose(0, 2, 1).reshape(C, S)
        stTP = np.ascontiguousarray(
            stTP.reshape(4, 128, S).transpose(1, 0, 2).reshape(128, 4 * S))
        in_maps.append({
            "ltT": ltTP, "stT": stTP, "wst": wst, "wlt": wlt, "wg": wg,
            "wcol": wcol, "bcol": bcol, "bst": bstv, "blt": bltv, "bg": bgv,
            "gam": gam, "bet": bet, "identh": identh,
        })
    return in_maps


def kernel(**inputs):
    from concourse.bass_utils import run_bass_kernel_spmd
    global LAST_EXEC_NS
    in_maps = prepare_in_maps(**inputs)

    nc = _get_program()
    trace = os.environ.get("BASS_KERNEL_TRACE", "") == "1"
    if trace:
        _install_ntff_shim()
    res = run_bass_kernel_spmd(nc, in_maps, core_ids=list(range(NB)),
                               trace=trace)
    LAST_EXEC_NS = res.exec_time_ns
    _CACHE["res"] = res
    out = np.empty((NB, D, S, C), np.float32)
    for n in range(NB):
        # device layout: out_dev[k, 256*d + s]
        out[n] = res.results[n]["out"].reshape(C, D, S).transpose(1, 2, 0)
    return out.reshape(NB, D, S, 1, C)



# revision 44
# speedup vs baseline: 1.6704x; 1.2897x over previous
"""Trainium2 Bass kernel for nn_NonLocalLayer (8-core data-parallel).

Math per batch n (see reference):
  theta = st @ w_st + b_st        (256,128)  -> reinterpret (128,256)  "theta_r"
  phi   = lt @ w_lt + b_lt        (4096,128) -> reinterpret (128,4096) "phi_r"
  g     = lt @ w_g  + b_g         (4096,128) -> reinterpret (128,4096) "g_r"
  attn  = theta_r^T @ phi_r / sqrt(128); p = softmax(attn, axis=l)
  out2  = g_r @ p^T               (128,256)
  y     = relu(LN(out2) * gamma + beta)      (128,256)
  out   = y[:, :, None]*w_out + b_out        (128,256,512)

Device strategy (per core = one batch):
  - host pre-transposes AND column-permutes st/lt (ltTP[c, m*128+i] =
    ltT[c, 32*i+m]) so every phi_r/g_r block is a contiguous matmul
  - big matmuls in fp16 (1 col/cyc on PE); accumulation fp32 in PSUM
  - softmax in transposed orientation (l on partitions) without
    max-subtraction (attn bounded ~ +-8); sums via ones-matmul over
    [1,512] pairs; normalization folded in after out2 accumulation
  - attention pipeline batched 2 l-blocks per stage (wider exp/copies)
  - epilogue: y flattened to one SBUF row (DMA), then out[k, d*256+s]
    = w[k]*yflat + b[k] as K=2 matmuls (lhsT = (w,b) col block, rhs =
    (yflat, ones) rows); PSUM->SBUF copies in f16 split DVE/ACT;
    OUTPUT IS STORED fp16 (tolerance 2e-2 >> f16 rounding 5e-4),
    halving the dominant HBM write traffic; host upcasts on gather
  - PE kept at 2.4 GHz (HAM warm): dummy matmuls during input loads
    and through the LayerNorm scalar chain avoid >3.4us PE-idle
    windows that would drop the clock gate to 1.2 GHz
"""
import math
import os

import numpy as np

NB = 8          # batch == n cores
S = 256         # NUM_ST
L = 4096        # NUM_LT
C = 512         # C_ST == C_LT
D = 128         # C_LAT
INV_SQRT_D = 1.0 / math.sqrt(float(D))
LN_EPS = 1e-3

_CACHE = {}
LAST_EXEC_NS = None


def _build_program():
    import concourse.bacc as bacc
    import concourse.tile as tile
    from concourse import bass_isa
    from concourse import mybir

    dt = mybir.dt
    F32 = dt.float32
    F16 = dt.float16
    BF16 = dt.bfloat16
    AF = mybir.ActivationFunctionType
    OP = mybir.AluOpType
    AX = mybir.AxisListType

    nc = bacc.Bacc("TRN2", target_bir_lowering=False, debug=False,
                   num_devices=NB)

    d_ltT = nc.dram_tensor("ltT", [128, 4 * L], F16, kind="ExternalInput")
    d_stT = nc.dram_tensor("stT", [128, 4 * S], F16, kind="ExternalInput")
    d_wst = nc.dram_tensor("wst", [128, 4 * D], F16, kind="ExternalInput")
    d_wlt = nc.dram_tensor("wlt", [128, 4 * D], F16, kind="ExternalInput")
    d_wg = nc.dram_tensor("wg", [128, 4 * D], F16, kind="ExternalInput")
    d_bst = nc.dram_tensor("bst", [1, D], F16, kind="ExternalInput")
    d_blt = nc.dram_tensor("blt", [D, 1], F32, kind="ExternalInput")
    d_bg = nc.dram_tensor("bg", [D, 1], F32, kind="ExternalInput")
    d_gam = nc.dram_tensor("gam", [D, S], F32, kind="ExternalInput")
    d_bet = nc.dram_tensor("bet", [D, S], F32, kind="ExternalInput")
    d_idh = nc.dram_tensor("identh", [128, 128], F16, kind="ExternalInput")
    d_wcol = nc.dram_tensor("wcol", [128, 4], F32, kind="ExternalInput")
    d_bcol = nc.dram_tensor("bcol", [128, 4], F32, kind="ExternalInput")
    d_out = nc.dram_tensor("out", [C, D * S], BF16, kind="ExternalOutput")

    with tile.TileContext(nc) as tc:
        # ---------- persistent pool (lives whole kernel) ----------
        with tc.tile_pool(name="keep", bufs=1) as keep:
            identh = keep.tile([128, 128], F16, tag="identh")
            bsth = keep.tile([1, D], F16, tag="bsth")
            blt_c = keep.tile([D, 1], F32, tag="blt_c")
            bg_c = keep.tile([D, 1], F32, tag="bg_c")
            gam = keep.tile([D, S], F32, tag="gam")
            bet = keep.tile([D, S], F32, tag="bet")
            wcol = keep.tile([128, 4], F32, tag="wcol")
            bcol = keep.tile([128, 4], F32, tag="bcol")
            yflat = keep.tile([1, D * S], BF16, tag="yflat")
            scr8 = keep.tile([1, 8], F32, tag="scr8")
            nc.vector.memset(scr8[:], 1.0)
            theta_r = keep.tile([128, S], F16, tag="theta_r")
            y_r = keep.tile([D, S], BF16, tag="y_r")
            warm = keep.tile([128, 512], F16, tag="warm")

            ones128 = keep.tile([128, 128], F16, tag="ones128")
            nc.vector.memset(ones128[:], 1.0)
            ones_f32 = keep.tile([128, 128], F32, tag="ones_f32")
            nc.vector.memset(ones_f32[:], 1.0)
            orow_h = keep.tile([1, 128], F16, tag="orow_h")
            nc.vector.memset(orow_h[:], 1.0)
            obh = keep.tile([1, 128], BF16, tag="obh")
            nc.vector.memset(obh[:], 1.0)
            nc.vector.memset(warm[:], 0.25)
            # preload the exp ACT table set while inputs stream in
            nc.scalar.activation(scr8[:, 0:1], scr8[:, 0:1], AF.Exp)

            # ---------- main phase ----------
            with tc.tile_pool(name="main", bufs=1) as main:
                # consolidated input tiles: all 4 k-blocks (j) side by
                # side, host-reordered so each 1MB DMA covers one lt
                # t-chunk for ALL j (fewer, bigger DMAs)
                ltt2 = main.tile([128, 4 * L], F16, tag="ltt2")
                stT2 = main.tile([128, 4 * S], F16, tag="stT2")
                wst2 = main.tile([128, 4 * D], F16, tag="wst2")
                wlt2 = main.tile([128, 4 * D], F16, tag="wlt2")
                wg2 = main.tile([128, 4 * D], F16, tag="wg2")

                def lt_sl(j, c0, c1):
                    # cols [c0,c1) of original ltTP[j]; c0,c1 within one
                    # 1024-col t-chunk
                    t = c0 // 1024
                    o = 4096 * t + 1024 * j + (c0 - 1024 * t)
                    return ltt2[:, o:o + (c1 - c0)]

                # loads round-robin over the two HWDGE queues (SWDGE/gpsimd
                # has ~1us setup + slow drain), ordered so the attention
                # pipeline can start ~5us in
                qs = [nc.sync, nc.scalar]
                qstate = [0]

                def ld(dst, src):
                    qs[qstate[0] % 2].dma_start(dst, src)
                    qstate[0] += 1

                ld(wlt2[:], d_wlt[:])
                ld(ltt2[:, 0:4096], d_ltT[:, 0:4096])
                ld(wg2[:], d_wg[:])
                ld(stT2[:], d_stT[:])
                ld(wst2[:], d_wst[:])
                ld(identh[:], d_idh[:])
                ld(bsth[:], d_bst[:])
                ld(blt_c[:], d_blt[:])
                ld(bg_c[:], d_bg[:])
                for t in range(1, 4):
                    ld(ltt2[:, 4096 * t:4096 * (t + 1)],
                       d_ltT[:, 4096 * t:4096 * (t + 1)])
                ld(gam[:], d_gam[:])
                ld(bet[:], d_bet[:])
                ld(wcol[:], d_wcol[:])
                ld(bcol[:], d_bcol[:])

                phiP = main.tile([D, L], F16, tag="phiP")
                gP = main.tile([D, L], F16, tag="gP")

                with tc.tile_pool(name="psL", bufs=1, space="PSUM") as psL, \
                     tc.tile_pool(name="loop", bufs=1) as lp:
                    # warm the PE clock gate while inputs stream in
                    for w in range(7):
                        pw = psL.tile([128, 512], F32, tag="att", bufs=2,
                                      name=f"pw{w}")
                        nc.tensor.matmul(pw[:], warm[:, 0:128], warm[:],
                                         start=True, stop=True)

                    p_out2 = psL.tile([D, S], F32, tag="acc")
                    p_sums = psL.tile([128, 2 * S], F32, tag="sums")

                    def emit_theta():
                        for h in range(2):
                            pth = psL.tile([128, D], F32, tag="att", bufs=2,
                                           name=f"pth{h}")
                            for j in range(4):
                                nc.tensor.matmul(
                                    pth[:],
                                    stT2[:, 256 * j + 128 * h:
                                         256 * j + 128 * (h + 1)],
                                    wst2[:, 128 * j:128 * (j + 1)],
                                    start=(j == 0), stop=False)
                            nc.tensor.matmul(pth[:], orow_h[:],
                                             bsth[:], start=False, stop=True)
                            nc.vector.tensor_copy(
                                theta_r[:, 128 * h:128 * (h + 1)], pth[:])

                    def emit_slice(sl):
                        cols = slice(512 * sl, 512 * (sl + 1))
                        for di, (dst, wts, bias_t) in enumerate(
                                ((phiP, wlt2, blt_c), (gP, wg2, bg_c))):
                            pmm = psL.tile([D, 512], F32, tag="mm", bufs=2,
                                           name=f"pmm{sl}_{di}")
                            for j in range(4):
                                nc.tensor.matmul(
                                    pmm[:],
                                    wts[:, 128 * j:128 * (j + 1)],
                                    lt_sl(j, 512 * sl, 512 * (sl + 1)),
                                    start=(j == 0), stop=(j == 3))
                            if di == 0:
                                nc.vector.tensor_scalar(
                                    dst[:, cols], pmm[:], bias_t[:, 0:1],
                                    None, OP.add)
                            else:
                                nc.scalar.activation(dst[:, cols], pmm[:],
                                                     AF.Identity,
                                                     bias=bias_t[:, 0:1])

                    ers = {}
                    for it in range(18):
                        if it % 2 == 0 and it // 2 < 8:
                            emit_slice(it // 2)
                        if it == 0:
                            emit_theta()
                        # stage A: transpose 2 phi blocks, attn matmuls, exp
                        if 1 <= it <= 16:
                            u = it - 1
                            ptp = psL.tile([128, 256], F16, tag="ptp", bufs=2,
                                           name=f"ptp{u}")
                            for i in range(2):
                                m = 2 * u + i
                                nc.tensor.transpose(
                                    ptp[:, 128 * i:128 * (i + 1)],
                                    phiP[:, 128 * m:128 * (m + 1)],
                                    identh[:])
                            phiR = lp.tile([128, 256], F16, tag="phiR",
                                           bufs=3, name=f"phiR{u}")
                            nc.vector.tensor_copy(phiR[:], ptp[:])
                            p_att = psL.tile([128, 512], F32, tag="att",
                                             bufs=2, name=f"patt{u}")
                            for i in range(2):
                                nc.tensor.matmul(
                                    p_att[:, 256 * i:256 * (i + 1)],
                                    phiR[:, 128 * i:128 * (i + 1)],
                                    theta_r[:], start=True, stop=True)
                            er = lp.tile([128, 512], F16, tag="er", bufs=3,
                                         name=f"er{u}")
                            nc.scalar.activation(er[:], p_att[:], AF.Exp,
                                                 scale=INV_SQRT_D)
                            ers[u] = er
                        # stage B: accumulate out2 and softmax sums
                        if 2 <= it <= 17:
                            u = it - 2
                            er = ers.pop(u)
                            for i in range(2):
                                m = 2 * u + i
                                nc.tensor.matmul(
                                    p_out2[:],
                                    gP[:, 128 * m:128 * (m + 1)],
                                    er[:, 256 * i:256 * (i + 1)],
                                    start=(m == 0), stop=(m == 31))
                            nc.tensor.matmul(p_sums[:], ones128[:], er[:],
                                             start=(u == 0), stop=(u == 15))

                    # softmax denominators (pre-broadcast: ones128 sums mm
                    # already produced identical rows on all 128 partitions)
                    sums_b = main.tile([128, 2 * S], F32, tag="sums_b")
                    nc.scalar.activation(sums_b[:], p_sums[:], AF.Identity)
                    # preload the sqrt ACT table set while DVE works below
                    nc.scalar.activation(scr8[:, 1:2], scr8[:, 0:1], AF.Sqrt)
                    zf = main.tile([128, S], F32, tag="zf")
                    nc.vector.tensor_tensor(zf[:], sums_b[:, 0:S],
                                            sums_b[:, S:2 * S], OP.add)
                    recip = main.tile([128, S], F32, tag="recip")
                    nc.vector.reciprocal(recip[:], zf[:])
                    # x = out2/Z, with per-partition sums of x and x^2
                    xt = main.tile([D, S], F32, tag="xt")
                    xsq = main.tile([D, S], F32, tag="xsq")
                    rs = main.tile([128, 2], F32, tag="rs")
                    nc.vector.tensor_tensor(xt[:], p_out2[:], recip[:],
                                            OP.mult)
                    nc.vector.tensor_tensor(xsq[:], xt[:], xt[:], OP.mult)
                    nc.vector.reduce_sum(rs[:, 0:1], xt[:], axis=AX.X)
                    nc.vector.reduce_sum(rs[:, 1:2], xsq[:], axis=AX.X)
                    # LN stats, kept per-partition (no broadcasts needed);
                    # partition reduction via tiny f32 ones-matmul (every
                    # output row gets the full sum)
                    p_rsum = psL.tile([128, 2], F32, tag="att", bufs=2,
                                      name="prsum")
                    nc.tensor.matmul(p_rsum[:], ones_f32[:], rs[:],
                                     start=True, stop=True)
                    rsum = main.tile([128, 2], F32, tag="rsum")
                    nc.vector.tensor_copy(rsum[:], p_rsum[:])
                    stat2 = main.tile([128, 2], F32, tag="stat2")
                    nc.vector.tensor_scalar(stat2[:], rsum[:],
                                            1.0 / (D * S), None, OP.mult)
                    m2 = main.tile([128, 1], F32, tag="m2")
                    nc.vector.tensor_tensor(m2[:], stat2[:, 0:1],
                                            stat2[:, 0:1], OP.mult)
                    vare = main.tile([128, 1], F32, tag="vare")
                    nc.vector.tensor_scalar(vare[:], stat2[:, 1:2],
                                            m2[:], LN_EPS,
                                            OP.subtract, OP.add)
                    sqv = main.tile([128, 1], F32, tag="sqv")
                    nc.scalar.activation(sqv[:], vare[:], AF.Sqrt)
                    rstd = main.tile([128, 1], F32, tag="rstd")
                    nc.vector.reciprocal(rstd[:], sqv[:])
                    t1 = main.tile([D, S], F32, tag="t1")
                    nc.vector.tensor_scalar(t1[:], xt[:],
                                            stat2[:, 0:1],
                                            rstd[:], OP.subtract, OP.mult)
                    t2 = main.tile([D, S], F32, tag="t2")
                    nc.vector.tensor_tensor(t2[:], t1[:], gam[:], OP.mult)
                    y = main.tile([D, S], F32, tag="y")
                    nc.vector.tensor_tensor(y[:], t2[:], bet[:], OP.add)
                    nc.vector.tensor_scalar_max(y_r[:], y[:], 0.0)
                    # flatten y (relu'd, f16) into one row: col = 256*d + s
                    nc.sync.dma_start(yflat[:], y_r[:])

            # ---------- epilogue: out[k, 256d+s] = w[k]*y[d,s] + b[k] ------
            # gpsimd broadcasts y to all partitions (idle engine, no PSUM);
            # DVE tensor_scalar (4x f16 mode) + ACT activation(scale,bias)
            # produce output stages directly -- no PE, no PSUM copies
            # ---------- epilogue: out[k, 256d+s] = w[k]*y[d,s] + b[k] ------
            # PE K=1 ones-matmul broadcasts y into PSUM (gpsimd broadcast
            # contends with DVE on SBUF ports -- avoid); DVE copies to a
            # rotating SBUF buffer; scale ops then run in fast SBUF mode:
            # DVE tensor_scalar (kb even) + ACT activation (kb odd)
            with tc.tile_pool(name="epi", bufs=1) as ep, \
                 tc.tile_pool(name="psE", bufs=1, space="PSUM") as psE:
                tidx = 0
                for t in range(4):
                    if t > 0:
                        sts = [ep.tile([128, 4096], BF16, tag=f"stg{kb}",
                                       bufs=2, name=f"st{t}_{kb}")
                               for kb in range(4)]
                    for gq in range(4):
                        g = 4 * t + gq
                        base = 2048 * g
                        pyb = psE.tile([128, 2048], F32, tag="pyb", bufs=2,
                                       name=f"pyb{g}")
                        for c in range(4):
                            nc.tensor.matmul(
                                pyb[:, 512 * c:512 * (c + 1)], obh[:],
                                yflat[:, base + 512 * c:base + 512 * (c + 1)],
                                start=True, stop=True)
                        ybuf = ep.tile([128, 2048], BF16, tag="ybuf",
                                       bufs=4, name=f"yb{g}")
                        nc.vector.tensor_copy(ybuf[:], pyb[:])
                        for kb in range(4):
                            wk = wcol[:, kb:kb + 1]
                            bk = bcol[:, kb:kb + 1]
                            if t == 0:
                                mst = ep.tile([128, 2048], BF16, tag="mst",
                                              bufs=6, name=f"mst{g}_{kb}")
                                dsl = mst[:]
                            else:
                                dsl = sts[kb][:, 2048 * (gq % 2):
                                              2048 * (gq % 2 + 1)]
                            if kb % 2 == 1:
                                nc.scalar.activation(dsl, ybuf[:],
                                                     AF.Identity,
                                                     bias=bk, scale=wk)
                            else:
                                nc.vector.tensor_scalar(dsl, ybuf[:], wk, bk,
                                                        OP.mult, OP.add)
                            if t == 0:
                                qeng = (nc.sync if tidx % 2 == 0
                                        else nc.scalar)
                                qeng.dma_start(
                                    d_out[128 * kb:128 * (kb + 1),
                                          base:base + 2048], mst[:])
                                tidx += 1
                        if t > 0 and gq % 2 == 1:
                            for kb in range(4):
                                qeng = (nc.sync if tidx % 2 == 0
                                        else nc.scalar)
                                qeng.dma_start(
                                    d_out[128 * kb:128 * (kb + 1),
                                          base - 2048:base + 2048],
                                    sts[kb][:])
                                tidx += 1
                            if gq == 1:
                                sts = [ep.tile([128, 4096], BF16,
                                               tag=f"stg{kb}", bufs=2,
                                               name=f"st{t}b_{kb}")
                                       for kb in range(4)]

    nc.compile()
    return nc


def _get_program():
    if "nc" not in _CACHE:
        _CACHE["nc"] = _build_program()
    return _CACHE["nc"]


def _install_ntff_shim():
    """Provide antenv.axon_hooks (absent in this image) so trace=True can
    capture NTFF profiles through the axon .so. Best-effort."""
    import sys
    import types
    try:
        from antenv.axon_hooks import get_axon_ntff_profile_hook  # noqa
        return
    except ImportError:
        pass
    try:
        from trn_agent_boot.trn_boot import _ntff_profile_via_ctypes
        hook = _ntff_profile_via_ctypes("/opt/axon/libaxon_pjrt.so")
        mod = types.ModuleType("antenv.axon_hooks")
        state = {"h": hook}
        mod.set_axon_ntff_profile_hook = lambda h: state.__setitem__("h", h)
        mod.get_axon_ntff_profile_hook = lambda: state["h"]
        sys.modules["antenv.axon_hooks"] = mod
        import antenv
        antenv.axon_hooks = mod
    except Exception as e:  # profiling is optional
        print(f"ntff shim unavailable: {e}")


def prepare_in_maps(st_feat, lt_feat, w_st, b_st, w_lt, b_lt, w_g, b_g,
                    ln_gamma, ln_beta, w_out, b_out):
    st_feat = np.asarray(st_feat, dtype=np.float32)
    lt_feat = np.asarray(lt_feat, dtype=np.float32)

    def wpack(w):
        w = np.asarray(w, np.float32).astype(np.float16)
        return np.ascontiguousarray(
            w.reshape(4, 128, 128).transpose(1, 0, 2).reshape(128, 512))

    wst = wpack(w_st)
    wlt = wpack(w_lt)
    wg = wpack(w_g)
    wcol = np.ascontiguousarray(
        np.asarray(w_out, np.float32).reshape(4, 128).T)
    bcol = np.ascontiguousarray(
        np.asarray(b_out, np.float32).reshape(4, 128).T)
    gam = np.ascontiguousarray(np.asarray(ln_gamma, np.float32)
                               .reshape(D, S))
    bet = np.ascontiguousarray(np.asarray(ln_beta, np.float32).reshape(D, S))
    bstv = np.asarray(b_st, np.float32).astype(np.float16).reshape(1, D)
    bltv = np.asarray(b_lt, np.float32).reshape(D, 1)
    bgv = np.asarray(b_g, np.float32).reshape(D, 1)
    identh = np.eye(128, dtype=np.float16)

    in_maps = []
    for n in range(NB):
        # column-permuted transposes: ltTP[c, m*128 + i] = ltT[c, 32*i + m]
        # and stTP[c, h*128 + i] = stT[c, 2*i + h]
        ltT = lt_feat[n].reshape(L, C).T.astype(np.float16)
        ltTP = ltT.reshape(C, 128, 32).transpose(0, 2, 1).reshape(C, L)
        # pack: lt2[p, 4096t + 1024j + i] = ltTP[128j + p, 1024t + i]
        ltTP = np.ascontiguousarray(
            ltTP.reshape(4, 128, 4, 1024).transpose(1, 2, 0, 3)
            .reshape(128, 4 * L))
        stT = st_feat[n].reshape(S, C).T.astype(np.float16)
        stTP = stT.reshape(C, 128, 2).transpose(0, 2, 1).reshape(C, S)
        stTP = np.ascontiguousarray(
            stTP.reshape(4, 128, S).transpose(1, 0, 2).reshape(128, 4 * S))
        in_maps.append({
            "ltT": ltTP, "stT": stTP, "wst": wst, "wlt": wlt, "wg": wg,
            "wcol": wcol, "bcol": bcol, "bst": bstv, "blt": bltv, "bg": bgv,
            "gam": gam, "bet": bet, "identh": identh,
        })
    return in_maps


def kernel(**inputs):
    from concourse.bass_utils import run_bass_kernel_spmd
    global LAST_EXEC_NS
    in_maps = prepare_in_maps(**inputs)

    nc = _get_program()
    trace = os.environ.get("BASS_KERNEL_TRACE", "") == "1"
    if trace:
        _install_ntff_shim()
    res = run_bass_kernel_spmd(nc, in_maps, core_ids=list(range(NB)),
                               trace=trace)
    LAST_EXEC_NS = res.exec_time_ns
    _CACHE["res"] = res
    out = np.empty((NB, D, S, C), np.float32)
    for n in range(NB):
        # device layout: out_dev[k, 256*d + s]
        out[n] = res.results[n]["out"].reshape(C, D, S).transpose(1, 2, 0)
    return out.reshape(NB, D, S, 1, C)

